# revision 4
# baseline (speedup 1.0000x reference)
"""Additive (Bahdanau) attention kernel for Trainium2, SPMD over 8 NeuronCores.

Problem: B=32, Q=1, V=4096, H=1024 (fp32).
  kp    = key @ Wk^T                      (B, V, H)
  h     = tanh(kp + query @ Wq^T + bias)  (B, V, H)
  score = h @ Ws^T + bs                   (B, V)
  attn  = softmax(score)                  (B, V)   [bs drops: softmax shift-invariant]
  ctx   = attn @ value                    (B, 1, H)

Sharding: data-parallel over batch, 4 batches per core, no collectives.

Per-core dataflow (all matmuls bf16 with fp32 PSUM accumulation):
  - kp^T computed in PSUM blocks [o=128, v=512]: lhsT = WkT chunk, rhs = keyT
    (key is transposed to [H, V] on the host so the contraction dim (h) is the
    partition dim for both operands).
  - tanh fused on ScalarE with per-partition bias qb = query@Wq^T + bias
    (precomputed on host, (B,H) - tiny), output bf16 tiles h_t [o=128, v=512].
  - score[v] accumulated on PE: psum[1, 512] += Ws_chunk^T @ h_t over 8 o-chunks.
  - softmax on the [1, 4096] row: reduce_max (negated) -> Exp with bias=-max and
    fused accum_out sum -> reciprocal -> scale.
  - attn reshaped to [128, 32] (v on partitions) via a DRAM round-trip, cast to
    bf16, then ctx[o] accumulated on PE: psum[1, o] += attn_chunk^T @ value_chunk
    over 32 v-chunks of 128 (value kept in natural [v, o] layout).
"""

import sys
import types

import numpy as np
import ml_dtypes

import concourse.bacc as bacc
import concourse.bass as bass
import concourse.mybir as mybir
import concourse.tile as tile
from concourse.bass_utils import run_bass_kernel_spmd

BF16 = mybir.dt.bfloat16
F32 = mybir.dt.float32
NP_BF16 = ml_dtypes.bfloat16

N_CORES = 8
B, Q, V, H = 32, 1, 4096, 1024
B_LOC = B // N_CORES          # 4 batches per core
VC = 512                      # v-chunk width for pass 1 (PSUM bank = 512 fp32)
N_VC = V // VC                # 8 v-chunks
N_OC = H // 128               # 8 o-chunks (output feature chunks)
N_HC = H // 128               # 8 h-chunks (contraction chunks)
N_CC = V // 128               # 32 v-chunks of 128 for the context matmul
QUARTER = V // 4              # keyT streamed in quarters of the v axis


def build_kernel():
    nc = bacc.Bacc("TRN2", target_bir_lowering=False, debug=False,
                   num_devices=N_CORES)

    keyT_d = nc.declare_dram_parameter("keyT", [B_LOC, H, V], BF16, isOutput=False)
    val_d = nc.declare_dram_parameter("value", [B_LOC, V, H], BF16, isOutput=False)
    wkT_d = nc.declare_dram_parameter("wkT", [H, H], BF16, isOutput=False)
    ws_d = nc.declare_dram_parameter("ws", [128, N_OC], BF16, isOutput=False)
    qb_d = nc.declare_dram_parameter("qb", [128, B_LOC * N_OC], F32, isOutput=False)
    ctx_d = nc.declare_dram_parameter("ctx_out", [B_LOC, H], F32, isOutput=True)
    attn_d = nc.declare_dram_parameter("attn_out", [B_LOC, V], F32, isOutput=True)

    with tile.TileContext(nc) as tc:
        with (
            tc.tile_pool(name="weights", bufs=1) as wpool,
            tc.tile_pool(name="keyq", bufs=2) as kpool,
            tc.tile_pool(name="vals", bufs=1) as vpool,
            tc.tile_pool(name="ht", bufs=2 * N_OC) as htpool,
            tc.tile_pool(name="rows", bufs=1) as rowpool,
            tc.tile_pool(name="small", bufs=2) as spool,
            tc.tile_pool(name="stats", bufs=4) as stpool,
            tc.tile_pool(name="pdram", bufs=2, space="DRAM") as dpool,
            tc.tile_pool(name="pskp", bufs=3, space="PSUM") as pskp,
            tc.tile_pool(name="pssc", bufs=2, space="PSUM") as pssc,
            tc.tile_pool(name="psctx", bufs=1, space="PSUM") as psctx,
        ):
            # ---- persistent weights ----
            wk_sb = wpool.tile([128, N_HC, H], BF16, tag="wk")
            nc.sync.dma_start(
                wk_sb[:], wkT_d[:].rearrange("(k p) o -> p k o", p=128))
            ws_sb = wpool.tile([128, N_OC], BF16, tag="ws")
            nc.sync.dma_start(ws_sb[:], ws_d[:])
            qb_sb = wpool.tile([128, B_LOC * N_OC], F32, tag="qb")
            nc.sync.dma_start(qb_sb[:], qb_d[:])

            key_tiles = {}   # (b, quarter) -> tile [128, N_HC, QUARTER]
            val_tiles = {}   # b -> tile [128, N_CC, H]

            def load_key_quarter(b, qtr):
                t = kpool.tile([128, N_HC, QUARTER], BF16, tag="keyq")
                src = keyT_d[b].rearrange("(k p) v -> p k v", p=128)
                nc.sync.dma_start(
                    t[:], src[:, :, qtr * QUARTER:(qtr + 1) * QUARTER])
                key_tiles[(b, qtr)] = t

            def load_value(b):
                t = vpool.tile([128, N_CC, H], BF16, tag="val")
                nc.gpsimd.dma_start(
                    t[:], val_d[b].rearrange("(c p) o -> p c o", p=128))
                val_tiles[b] = t

            score_sb = {}    # b -> [1, V] f32

            def emit_kp_chunk(b, j):
                """64 matmuls + 8 tanh for v-chunk j of batch b. Returns h_t tiles."""
                per_q = QUARTER // VC
                kt = key_tiles[(b, j // per_q)]
                v0 = (j % per_q) * VC
                hts = []
                for i in range(N_OC):
                    ps = pskp.tile([128, VC], F32, tag="pskp")
                    for k in range(N_HC):
                        nc.tensor.matmul(
                            ps[:],
                            wk_sb[:, k, i * 128:(i + 1) * 128],
                            kt[:, k, v0:v0 + VC],
                            start=(k == 0), stop=(k == N_HC - 1))
                    ht = htpool.tile([128, VC], BF16, tag="ht")
                    nc.scalar.activation(
                        ht[:], ps[:], mybir.ActivationFunctionType.Tanh,
                        bias=qb_sb[:, b * N_OC + i:b * N_OC + i + 1])
                    hts.append(ht)
                return hts

            def emit_score_chunk(b, j, hts):
                ps = pssc.tile([1, VC], F32, tag="pssc")
                for i in range(N_OC):
                    nc.tensor.matmul(
                        ps[:], ws_sb[:, i:i + 1], hts[i][:],
                        start=(i == 0), stop=(i == N_OC - 1))
                if b not in score_sb:
                    score_sb[b] = rowpool.tile([1, V], F32, tag="score", name="score")
                nc.vector.tensor_copy(score_sb[b][0:1, j * VC:(j + 1) * VC], ps[:])

            attn_rs = {}     # b -> [128, N_CC] bf16 (v on partitions)

            def emit_softmax(b):
                mx = stpool.tile([1, 1], F32, tag="mx")
                nc.vector.reduce_max(
                    mx[:], score_sb[b][:], axis=mybir.AxisListType.X, negate=True)
                p = rowpool.tile([1, V], F32, tag="p")
                sm = stpool.tile([1, 1], F32, tag="sm")
                nc.scalar.activation(
                    p[:], score_sb[b][:], mybir.ActivationFunctionType.Exp,
                    bias=mx[:], accum_out=sm[:])
                rs = stpool.tile([1, 1], F32, tag="rs")
                nc.vector.reciprocal(rs[:], sm[:])
                attn_f = rowpool.tile([1, V], F32, tag="attnf")
                nc.vector.tensor_scalar_mul(attn_f[:], p[:], rs[:])
                nc.scalar.dma_start(attn_d[b:b + 1, :], attn_f[0:1, :])
                # reshape [1, V] -> [128, N_CC] (v = c*128 + p) via DRAM
                # round-trip: scatter-write with permuted dims, read naturally.
                pd = dpool.tile([128, N_CC], F32, tag="pd")
                nc.scalar.dma_start(pd[:].rearrange("p c -> c p"), attn_f[0:1, :])
                prs = spool.tile([128, N_CC], F32, tag="prs")
                nc.scalar.dma_start(prs[:], pd[:])
                pbf = spool.tile([128, N_CC], BF16, tag="pbf")
                nc.vector.tensor_copy(pbf[:], prs[:])
                attn_rs[b] = pbf

            def emit_ctx(b):
                pbf = attn_rs[b]
                vt = val_tiles[b]
                ps = psctx.tile([1, H], F32, tag="psctx")
                for half in range(2):
                    o0 = half * 512
                    for c in range(N_CC):
                        nc.tensor.matmul(
                            ps[0:1, o0:o0 + 512],
                            pbf[:, c:c + 1],
                            vt[:, c, o0:o0 + 512],
                            start=(c == 0), stop=(c == N_CC - 1))
                ctx_sb = spool.tile([1, H], F32, tag="ctxsb")
                nc.vector.tensor_copy(ctx_sb[:], ps[:])
                nc.scalar.dma_start(ctx_d[b:b + 1, :], ctx_sb[0:1, :])

            # ---- main emission loop (PE program order is emission order) ----
            load_key_quarter(0, 0)
            load_key_quarter(0, 1)
            pending = None
            for b in range(B_LOC):
                for j in range(N_VC):
                    if j == 0:
                        load_key_quarter(b, 2)
                        load_key_quarter(b, 3)
                        load_value(b)
                    if j == 6 and b + 1 < B_LOC:
                        load_key_quarter(b + 1, 0)
                        load_key_quarter(b + 1, 1)
                    hts = emit_kp_chunk(b, j)
                    if pending is not None:
                        emit_score_chunk(*pending)
                    pending = (b, j, hts)
                    if j == 0 and b > 0:
                        emit_softmax(b - 1)
                    if j == 1 and b > 0:
                        emit_ctx(b - 1)
            emit_score_chunk(*pending)
            emit_softmax(B_LOC - 1)
            emit_ctx(B_LOC - 1)

    nc.finalize()
    return nc


_NC_CACHE = None


def _get_nc():
    global _NC_CACHE
    if _NC_CACHE is None:
        _NC_CACHE = build_kernel()
    return _NC_CACHE


def prep_inputs(query, key, value, Wq, Wk, bias, Ws, bs):
    """Host-side shard + layout prep. Returns in_maps for the 8 cores."""
    query = np.asarray(query, dtype=np.float32)
    key = np.asarray(key, dtype=np.float32)
    value = np.asarray(value, dtype=np.float32)
    Wq = np.asarray(Wq, dtype=np.float32)
    Wk = np.asarray(Wk, dtype=np.float32)
    bias = np.asarray(bias, dtype=np.float32)
    Ws = np.asarray(Ws, dtype=np.float32)

    wkT = np.ascontiguousarray(Wk.T).astype(NP_BF16)            # [h, o]
    ws_p = np.ascontiguousarray(Ws.reshape(N_OC, 128).T).astype(NP_BF16)
    qb_full = (query[:, 0, :] @ Wq.T + bias).astype(np.float32)  # [B, H]

    in_maps = []
    for c in range(N_CORES):
        sl = slice(c * B_LOC, (c + 1) * B_LOC)
        keyT = key[sl].transpose(0, 2, 1).astype(NP_BF16)        # [B_LOC, H, V]
        val_b = value[sl].astype(NP_BF16)                        # [B_LOC, V, H]
        qb = np.ascontiguousarray(
            qb_full[sl].reshape(B_LOC, N_OC, 128).transpose(2, 0, 1)
            .reshape(128, B_LOC * N_OC))
        in_maps.append({
            "keyT": np.ascontiguousarray(keyT),
            "value": np.ascontiguousarray(val_b),
            "wkT": wkT,
            "ws": ws_p,
            "qb": qb,
        })
    return in_maps


def kernel(query, key, value, Wq, Wk, bias, Ws, bs):
    nc = _get_nc()
    in_maps = prep_inputs(query, key, value, Wq, Wk, bias, Ws, bs)
    res = run_bass_kernel_spmd(nc, in_maps, core_ids=list(range(N_CORES)))
    context = np.empty((B, Q, H), dtype=np.float32)
    attn = np.empty((B, V), dtype=np.float32)
    for c in range(N_CORES):
        sl = slice(c * B_LOC, (c + 1) * B_LOC)
        context[sl, 0, :] = res.results[c]["ctx_out"]
        attn[sl] = res.results[c]["attn_out"]
    return context, attn


# revision 6
# speedup vs baseline: 1.1358x; 1.1358x over previous
"""Additive (Bahdanau) attention kernel for Trainium2, SPMD over 8 NeuronCores.

Problem: B=32, Q=1, V=4096, H=1024 (fp32).
  kp    = key @ Wk^T                      (B, V, H)
  h     = tanh(kp + query @ Wq^T + bias)  (B, V, H)
  score = h @ Ws^T + bs                   (B, V)
  attn  = softmax(score)                  (B, V)   [bs drops: softmax shift-invariant]
  ctx   = attn @ value                    (B, 1, H)

Sharding: data-parallel over batch, 4 batches per core, no collectives.

Per-core dataflow (all matmuls bf16 with fp32 PSUM accumulation):
  - kp^T computed in PSUM blocks [o=128, v=512]: lhsT = WkT chunk, rhs = keyT
    (key is transposed to [H, V] on the host so the contraction dim (h) is the
    partition dim for both operands). Loop order (i, k, jj) shares each
    LDWEIGHTS across two 512-wide matmuls.
  - tanh fused on ScalarE with per-partition bias qb = query@Wq^T + bias
    (precomputed on host, (B,H) - tiny), output bf16 tiles h_t [o=128, v=512].
  - score[v] accumulated on PE: psum[1, 512] += Ws_chunk^T @ h_t over 8 o-chunks.
  - softmax on the [1, 4096] row: reduce_max (negated) -> Exp with bias=-max and
    fused accum_out sum -> reciprocal.
  - unnormalized p written contiguously to a DRAM scratch tile, read back as
    [32, 128], PE-transposed to [128, 32] (v on partitions), cast to bf16;
    ctx_raw[o] accumulated on PE: psum[1, o] += p_chunk^T @ value_chunk over
    32 v-chunks of 128 (value in natural [v, o] layout); final 1/sum scale is
    fused into the PSUM->SBUF copy.
"""

import numpy as np
import ml_dtypes

import concourse.bacc as bacc
import concourse.bass as bass
import concourse.mybir as mybir
import concourse.tile as tile
from concourse.bass_utils import run_bass_kernel_spmd

BF16 = mybir.dt.bfloat16
F32 = mybir.dt.float32
NP_BF16 = ml_dtypes.bfloat16

N_CORES = 8
B, Q, V, H = 32, 1, 4096, 1024
B_LOC = B // N_CORES          # 4 batches per core
VC = 512                      # v-chunk width for pass 1 (PSUM bank = 512 fp32)
N_VC = V // VC                # 8 v-chunks
N_OC = H // 128               # 8 o-chunks (output feature chunks)
N_HC = H // 128               # 8 h-chunks (contraction chunks)
N_CC = V // 128               # 32 v-chunks of 128 for the context matmul
QUARTER = V // 4              # keyT streamed in quarters of the v axis
PAIR = 2 * VC                 # v-chunk pair (one LDWEIGHTS serves both)


def build_kernel():
    nc = bacc.Bacc("TRN2", target_bir_lowering=False, debug=False,
                   num_devices=N_CORES)

    keyT_d = nc.declare_dram_parameter("keyT", [B_LOC, H, V], BF16, isOutput=False)
    val_d = nc.declare_dram_parameter("value", [B_LOC, V, H], BF16, isOutput=False)
    wkT_d = nc.declare_dram_parameter("wkT", [H, H], BF16, isOutput=False)
    ws_d = nc.declare_dram_parameter("ws", [128, N_OC], BF16, isOutput=False)
    qb_d = nc.declare_dram_parameter("qb", [128, B_LOC * N_OC], F32, isOutput=False)
    id_d = nc.declare_dram_parameter("ident", [32, 32], F32, isOutput=False)
    ctx_d = nc.declare_dram_parameter("ctx_out", [B_LOC, H], F32, isOutput=True)
    attn_d = nc.declare_dram_parameter("attn_out", [B_LOC, V], F32, isOutput=True)

    with tile.TileContext(nc) as tc:
        with (
            tc.tile_pool(name="weights", bufs=1) as wpool,
            tc.tile_pool(name="keyq", bufs=2) as kpool,
            tc.tile_pool(name="vals", bufs=1) as vpool,
            tc.tile_pool(name="ht", bufs=4 * N_OC) as htpool,
            tc.tile_pool(name="rows", bufs=1) as rowpool,
            tc.tile_pool(name="small", bufs=2) as spool,
            tc.tile_pool(name="stats", bufs=4) as stpool,
            tc.tile_pool(name="pdram", bufs=2, space="DRAM") as dpool,
            tc.tile_pool(name="pskp", bufs=4, space="PSUM") as pskp,
            tc.tile_pool(name="pssc", bufs=1, space="PSUM") as pssc,
            tc.tile_pool(name="pstr", bufs=1, space="PSUM") as pstr,
            tc.tile_pool(name="psctx", bufs=1, space="PSUM") as psctx,
        ):
            # ---- persistent weights (scalar-engine DMA queue, so the big
            # sync-queue keyT loads don't serialize behind them) ----
            wk_sb = wpool.tile([128, N_HC, H], BF16, tag="wk")
            nc.scalar.dma_start(
                wk_sb[:], wkT_d[:].rearrange("(k p) o -> p k o", p=128))
            ws_sb = wpool.tile([128, N_OC], BF16, tag="ws")
            nc.scalar.dma_start(ws_sb[:], ws_d[:])
            qb_sb = wpool.tile([128, B_LOC * N_OC], F32, tag="qb")
            nc.scalar.dma_start(qb_sb[:], qb_d[:])
            id_sb = wpool.tile([32, 32], F32, tag="ident")
            nc.scalar.dma_start(id_sb[:], id_d[:])

            key_tiles = {}   # (b, quarter) -> tile [128, N_HC, QUARTER]
            val_tiles = {}   # b -> tile [128, N_CC, H]

            def load_key_quarter(b, qtr):
                t = kpool.tile([128, N_HC, QUARTER], BF16, tag="keyq",
                               name="keyq")
                src = keyT_d[b].rearrange("(k p) v -> p k v", p=128)
                nc.sync.dma_start(
                    t[:], src[:, :, qtr * QUARTER:(qtr + 1) * QUARTER])
                key_tiles[(b, qtr)] = t

            def load_value(b):
                t = vpool.tile([128, N_CC, H], BF16, tag="val", name="val")
                nc.gpsimd.dma_start(
                    t[:], val_d[b].rearrange("(c p) o -> p c o", p=128))
                val_tiles[b] = t

            score_sb = {}    # b -> [1, V] f32

            def emit_kp_pair(b, jp):
                """Pair of v-chunks (2*jp, 2*jp+1): 128 matmuls + 16 tanh.

                Loop (i, k, jj): one LDWEIGHTS of wk[k, i-chunk] feeds both
                512-wide matmuls of the pair. Returns h_t tiles keyed (i, jj).
                """
                kt = key_tiles[(b, jp // (QUARTER // PAIR))]
                v0 = (jp % (QUARTER // PAIR)) * PAIR
                hts = {}
                for i in range(N_OC):
                    pss = [pskp.tile([128, VC], F32, tag="pskp", name="pskp")
                           for _ in range(2)]
                    for k in range(N_HC):
                        for jj in range(2):
                            nc.tensor.matmul(
                                pss[jj][:],
                                wk_sb[:, k, i * 128:(i + 1) * 128],
                                kt[:, k, v0 + jj * VC:v0 + (jj + 1) * VC],
                                start=(k == 0), stop=(k == N_HC - 1))
                    for jj in range(2):
                        ht = htpool.tile([128, VC], BF16, tag="ht", name="ht")
                        nc.scalar.activation(
                            ht[:], pss[jj][:], mybir.ActivationFunctionType.Tanh,
                            bias=qb_sb[:, b * N_OC + i:b * N_OC + i + 1])
                        hts[(i, jj)] = ht
                return hts

            def emit_score_pair(b, jp, hts):
                for jj in range(2):
                    j = 2 * jp + jj
                    ps = pssc.tile([1, VC], F32, tag="pssc", name="pssc")
                    for i in range(N_OC):
                        nc.tensor.matmul(
                            ps[:], ws_sb[:, i:i + 1], hts[(i, jj)][:],
                            start=(i == 0), stop=(i == N_OC - 1))
                    if b not in score_sb:
                        score_sb[b] = rowpool.tile([1, V], F32, tag="score",
                                                   name="score")
                    nc.vector.tensor_copy(
                        score_sb[b][0:1, j * VC:(j + 1) * VC], ps[:])

            p_back = {}      # b -> ([32, 128] f32 tile, rs [1,1] tile)

            def emit_softmax(b):
                mx = stpool.tile([1, 1], F32, tag="mx", name="mx")
                nc.vector.reduce_max(
                    mx[:], score_sb[b][:], axis=mybir.AxisListType.X, negate=True)
                p = rowpool.tile([1, V], F32, tag="p", name="p")
                sm = stpool.tile([1, 1], F32, tag="sm", name="sm")
                nc.scalar.activation(
                    p[:], score_sb[b][:], mybir.ActivationFunctionType.Exp,
                    bias=mx[:], accum_out=sm[:])
                rs = stpool.tile([1, 1], F32, tag="rs", name="rs")
                nc.vector.reciprocal(rs[:], sm[:])
                # unnormalized p -> DRAM (contiguous), read back [32, 128]
                pd = dpool.tile([32, 128], F32, tag="pd", name="pd")
                nc.scalar.dma_start(
                    pd[:].rearrange("c p -> (c p)")[None, :], p[0:1, :])
                prs = spool.tile([32, 128], F32, tag="prs", name="prs")
                nc.scalar.dma_start(prs[:], pd[:])
                p_back[b] = (prs, rs)
                # normalized attn output
                attn_f = rowpool.tile([1, V], F32, tag="score", name="attnf")
                nc.vector.tensor_scalar_mul(attn_f[:], p[:], rs[:])
                nc.scalar.dma_start(attn_d[b:b + 1, :], attn_f[0:1, :])

            def emit_ctx(b):
                prs, rs = p_back[b]
                # transpose [32, 128] -> psum [128, 32] on PE
                pst = pstr.tile([128, 32], F32, tag="pstr", name="pstr")
                nc.tensor.transpose(pst[:], prs[:], id_sb[:])
                pbf = spool.tile([128, N_CC], BF16, tag="pbf", name="pbf")
                nc.vector.tensor_copy(pbf[:], pst[:])
                vt = val_tiles[b]
                ps = psctx.tile([1, H], F32, tag="psctx", name="psctx")
                for half in range(2):
                    o0 = half * 512
                    for c in range(N_CC):
                        nc.tensor.matmul(
                            ps[0:1, o0:o0 + 512],
                            pbf[:, c:c + 1],
                            vt[:, c, o0:o0 + 512],
                            start=(c == 0), stop=(c == N_CC - 1))
                ctx_sb = spool.tile([1, H], F32, tag="ctxsb", name="ctxsb")
                nc.vector.tensor_scalar_mul(ctx_sb[:], ps[:], rs[:])
                nc.scalar.dma_start(ctx_d[b:b + 1, :], ctx_sb[0:1, :])

            # ---- main emission loop (PE program order is emission order) ----
            N_PAIR = N_VC // 2
            load_key_quarter(0, 0)
            load_key_quarter(0, 1)
            pending = None
            for b in range(B_LOC):
                for jp in range(N_PAIR):
                    if jp == 0:
                        load_key_quarter(b, 2)
                        load_key_quarter(b, 3)
                        load_value(b)
                    if jp == 3 and b + 1 < B_LOC:
                        load_key_quarter(b + 1, 0)
                        load_key_quarter(b + 1, 1)
                    hts = emit_kp_pair(b, jp)
                    if pending is not None:
                        emit_score_pair(*pending)
                    pending = (b, jp, hts)
                    if jp == 0 and b > 0:
                        emit_softmax(b - 1)
                    if jp == 1 and b > 0:
                        emit_ctx(b - 1)
            emit_score_pair(*pending)
            emit_softmax(B_LOC - 1)
            emit_ctx(B_LOC - 1)

    nc.finalize()
    return nc


_NC_CACHE = None


def _get_nc():
    global _NC_CACHE
    if _NC_CACHE is None:
        _NC_CACHE = build_kernel()
    return _NC_CACHE


def prep_inputs(query, key, value, Wq, Wk, bias, Ws, bs):
    """Host-side shard + layout prep. Returns in_maps for the 8 cores."""
    query = np.asarray(query, dtype=np.float32)
    key = np.asarray(key, dtype=np.float32)
    value = np.asarray(value, dtype=np.float32)
    Wq = np.asarray(Wq, dtype=np.float32)
    Wk = np.asarray(Wk, dtype=np.float32)
    bias = np.asarray(bias, dtype=np.float32)
    Ws = np.asarray(Ws, dtype=np.float32)

    wkT = np.ascontiguousarray(Wk.T).astype(NP_BF16)            # [h, o]
    ws_p = np.ascontiguousarray(Ws.reshape(N_OC, 128).T).astype(NP_BF16)
    qb_full = (query[:, 0, :] @ Wq.T + bias).astype(np.float32)  # [B, H]
    ident = np.eye(32, dtype=np.float32)

    in_maps = []
    for c in range(N_CORES):
        sl = slice(c * B_LOC, (c + 1) * B_LOC)
        keyT = key[sl].transpose(0, 2, 1).astype(NP_BF16)        # [B_LOC, H, V]
        val_b = value[sl].astype(NP_BF16)                        # [B_LOC, V, H]
        qb = np.ascontiguousarray(
            qb_full[sl].reshape(B_LOC, N_OC, 128).transpose(2, 0, 1)
            .reshape(128, B_LOC * N_OC))
        in_maps.append({
            "keyT": np.ascontiguousarray(keyT),
            "value": np.ascontiguousarray(val_b),
            "wkT": wkT,
            "ws": ws_p,
            "qb": qb,
            "ident": ident,
        })
    return in_maps


def kernel(query, key, value, Wq, Wk, bias, Ws, bs):
    nc = _get_nc()
    in_maps = prep_inputs(query, key, value, Wq, Wk, bias, Ws, bs)
    res = run_bass_kernel_spmd(nc, in_maps, core_ids=list(range(N_CORES)))
    context = np.empty((B, Q, H), dtype=np.float32)
    attn = np.empty((B, V), dtype=np.float32)
    for c in range(N_CORES):
        sl = slice(c * B_LOC, (c + 1) * B_LOC)
        context[sl, 0, :] = res.results[c]["ctx_out"]
        attn[sl] = res.results[c]["attn_out"]
    return context, attn


# revision 13
# speedup vs baseline: 1.1540x; 1.0160x over previous
"""Additive (Bahdanau) attention kernel for Trainium2, SPMD over 8 NeuronCores.

Problem: B=32, Q=1, V=4096, H=1024 (fp32).
  kp    = key @ Wk^T                      (B, V, H)
  h     = tanh(kp + query @ Wq^T + bias)  (B, V, H)
  score = h @ Ws^T + bs                   (B, V)
  attn  = softmax(score)                  (B, V)   [bs drops: softmax shift-invariant]
  ctx   = attn @ value                    (B, 1, H)

Sharding: data-parallel over batch, 4 batches per core, no collectives.

Per-core dataflow (all matmuls bf16 with fp32 PSUM accumulation):
  - kp^T computed in PSUM blocks [o=128, v=512]: lhsT = WkT chunk, rhs = keyT
    (key is transposed to [H, V] on the host so the contraction dim (h) is the
    partition dim for both operands). Loop order (i, k, jj) shares each
    LDWEIGHTS across two 512-wide matmuls.
  - tanh fused on ScalarE with per-partition bias qb = query@Wq^T + bias
    (precomputed on host, (B,H) - tiny), output bf16 tiles h_t [o=128, v=512].
  - score[v] accumulated on PE: psum[1, 512] += Ws_chunk^T @ h_t over 8 o-chunks.
  - softmax on the [1, 4096] row: reduce_max (negated) -> Exp with bias=-max and
    fused accum_out sum -> reciprocal.
  - unnormalized p written contiguously to a DRAM scratch tile, read back as
    [32, 128], PE-transposed to [128, 32] (v on partitions), cast to bf16;
    ctx_raw[o] accumulated on PE: psum[1, o] += p_chunk^T @ value_chunk over
    32 v-chunks of 128 (value in natural [v, o] layout); final 1/sum scale is
    fused into the PSUM->SBUF copy.
"""

import numpy as np
import ml_dtypes

import concourse.bacc as bacc
import concourse.bass as bass
import concourse.mybir as mybir
import concourse.tile as tile
from concourse.bass_utils import run_bass_kernel_spmd

BF16 = mybir.dt.bfloat16
F32 = mybir.dt.float32
NP_BF16 = ml_dtypes.bfloat16

N_CORES = 8
B, Q, V, H = 32, 1, 4096, 1024
B_LOC = B // N_CORES          # 4 batches per core
VC = 512                      # v-chunk width for pass 1 (PSUM bank = 512 fp32)
N_VC = V // VC                # 8 v-chunks
N_OC = H // 128               # 8 o-chunks (output feature chunks)
N_HC = H // 128               # 8 h-chunks (contraction chunks)
N_CC = V // 128               # 32 v-chunks of 128 for the context matmul
QUARTER = V // 4              # keyT streamed in quarters of the v axis
PAIR = 2 * VC                 # v-chunk pair (one LDWEIGHTS serves both)


def build_kernel():
    nc = bacc.Bacc("TRN2", target_bir_lowering=False, debug=False,
                   num_devices=N_CORES)

    keyT_d = nc.declare_dram_parameter("keyT", [B_LOC, H, V], BF16, isOutput=False)
    val_d = nc.declare_dram_parameter("value", [B_LOC, V, H], BF16, isOutput=False)
    wkT_d = nc.declare_dram_parameter("wkT", [H, H], BF16, isOutput=False)
    ws_d = nc.declare_dram_parameter("ws", [128, N_OC], BF16, isOutput=False)
    qb_d = nc.declare_dram_parameter("qb", [128, B_LOC * N_OC], F32, isOutput=False)
    id_d = nc.declare_dram_parameter("ident", [32, 32], F32, isOutput=False)
    ctx_d = nc.declare_dram_parameter("ctx_out", [B_LOC, H], F32, isOutput=True)
    attn_d = nc.declare_dram_parameter("attn_out", [B_LOC, V], F32, isOutput=True)

    with tile.TileContext(nc) as tc:
        with (
            tc.tile_pool(name="weights", bufs=1) as wpool,
            tc.tile_pool(name="keyq", bufs=2) as kpool,
            tc.tile_pool(name="vals", bufs=1) as vpool,
            tc.tile_pool(name="ht", bufs=4 * N_OC) as htpool,
            tc.tile_pool(name="rows", bufs=1) as rowpool,
            tc.tile_pool(name="small", bufs=2) as spool,
            tc.tile_pool(name="stats", bufs=4) as stpool,
            tc.tile_pool(name="pdram", bufs=2, space="DRAM") as dpool,
            tc.tile_pool(name="pskp", bufs=3, space="PSUM") as pskp,
            tc.tile_pool(name="pssc", bufs=2, space="PSUM") as pssc,
            tc.tile_pool(name="pstr", bufs=1, space="PSUM") as pstr,
            tc.tile_pool(name="psctx", bufs=2, space="PSUM") as psctx,
        ):
            # ---- persistent weights (scalar-engine DMA queue, so the big
            # sync-queue keyT loads don't serialize behind them) ----
            wk_sb = wpool.tile([128, N_HC, H], BF16, tag="wk")
            nc.scalar.dma_start(
                wk_sb[:], wkT_d[:].rearrange("(k p) o -> p k o", p=128))
            ws_sb = wpool.tile([128, N_OC], BF16, tag="ws")
            nc.scalar.dma_start(ws_sb[:], ws_d[:])
            qb_sb = wpool.tile([128, B_LOC * N_OC], F32, tag="qb")
            nc.scalar.dma_start(qb_sb[:], qb_d[:])
            id_sb = wpool.tile([32, 32], F32, tag="ident")
            nc.scalar.dma_start(id_sb[:], id_d[:])

            key_tiles = {}   # (b, quarter) -> tile [128, N_HC, QUARTER]
            val_tiles = {}   # b -> tile [128, N_CC, H]

            def load_key_quarter(b, qtr):
                t = kpool.tile([128, N_HC, QUARTER], BF16, tag="keyq",
                               name="keyq")
                src = keyT_d[b].rearrange("(k p) v -> p k v", p=128)
                d = nc.sync.dma_start(
                    t[:], src[:, :, qtr * QUARTER:(qtr + 1) * QUARTER])
                key_tiles[(b, qtr)] = t
                return d

            def load_value(b):
                t = vpool.tile([128, N_CC, H], BF16, tag="val", name="val")
                d = nc.gpsimd.dma_start(
                    t[:], val_d[b].rearrange("(c p) o -> p c o", p=128))
                val_tiles[b] = t
                return d

            score_sb = {}    # b -> [1, V] f32

            def emit_kp_pair(b, jp):
                """Pair of v-chunks (2*jp, 2*jp+1): 128 matmuls + 16 tanh.

                Loop (i, k, jj): one LDWEIGHTS of wk[k, i-chunk] feeds both
                512-wide matmuls of the pair. Returns h_t tiles keyed (i, jj).
                """
                kt = key_tiles[(b, jp // (QUARTER // PAIR))]
                v0 = (jp % (QUARTER // PAIR)) * PAIR
                hts = {}
                last_mm = None
                for i in range(N_OC):
                    pss = [pskp.tile([128, VC], F32, tag="pskp", name="pskp")
                           for _ in range(2)]
                    for k in range(N_HC):
                        for jj in range(2):
                            last_mm = nc.tensor.matmul(
                                pss[jj][:],
                                wk_sb[:, k, i * 128:(i + 1) * 128],
                                kt[:, k, v0 + jj * VC:v0 + (jj + 1) * VC],
                                start=(k == 0), stop=(k == N_HC - 1))
                    for jj in range(2):
                        ht = htpool.tile([128, VC], BF16, tag="ht", name="ht")
                        nc.scalar.activation(
                            ht[:], pss[jj][:], mybir.ActivationFunctionType.Tanh,
                            bias=qb_sb[:, b * N_OC + i:b * N_OC + i + 1])
                        hts[(i, jj)] = ht
                return hts, last_mm

            pair_max = {}    # b -> [1, N_VC // 2] f32 per-pair running maxes

            def emit_score_pair(b, jp, hts):
                if b not in score_sb:
                    score_sb[b] = rowpool.tile([1, V], F32, tag="score",
                                               name="score")
                    pair_max[b] = stpool.tile([1, N_VC // 2], F32, tag="pmax",
                                              name="pmax")
                for jj in range(2):
                    j = 2 * jp + jj
                    ps = pssc.tile([1, VC], F32, tag="pssc", name="pssc")
                    for i in range(N_OC):
                        nc.tensor.matmul(
                            ps[:], ws_sb[:, i:i + 1], hts[(i, jj)][:],
                            start=(i == 0), stop=(i == N_OC - 1))
                    nc.vector.tensor_copy(
                        score_sb[b][0:1, j * VC:(j + 1) * VC], ps[:])
                # per-pair max (overlapped with later compute; shortens the
                # final softmax critical path)
                nc.vector.reduce_max(
                    pair_max[b][0:1, jp:jp + 1],
                    score_sb[b][0:1, jp * PAIR:(jp + 1) * PAIR],
                    axis=mybir.AxisListType.X)

            p_back = {}      # b -> ([32, 128] f32 tile, rs [1,1] tile)

            def emit_softmax(b):
                mx = stpool.tile([1, 1], F32, tag="mx", name="mx")
                nc.vector.reduce_max(
                    mx[:], pair_max[b][:], axis=mybir.AxisListType.X, negate=True)
                p = rowpool.tile([1, V], F32, tag="p", name="p")
                sm = stpool.tile([1, 1], F32, tag="sm", name="sm")
                nc.scalar.activation(
                    p[:], score_sb[b][:], mybir.ActivationFunctionType.Exp,
                    bias=mx[:], accum_out=sm[:])
                rs = stpool.tile([1, 1], F32, tag="rs", name="rs")
                nc.vector.reciprocal(rs[:], sm[:])
                # unnormalized p -> DRAM (contiguous), read back [32, 128]
                pd = dpool.tile([32, 128], F32, tag="pd", name="pd")
                nc.scalar.dma_start(
                    pd[:].rearrange("c p -> (c p)")[None, :], p[0:1, :])
                prs = spool.tile([32, 128], F32, tag="prs", name="prs")
                nc.scalar.dma_start(prs[:], pd[:])
                p_back[b] = (prs, rs)
                # normalized attn output
                attn_f = rowpool.tile([1, V], F32, tag="score", name="attnf")
                nc.vector.tensor_scalar_mul(attn_f[:], p[:], rs[:])
                nc.scalar.dma_start(attn_d[b:b + 1, :], attn_f[0:1, :])

            def emit_ctx(b):
                prs, rs = p_back[b]
                # transpose [32, 128] -> psum [128, 32] on PE
                pst = pstr.tile([128, 32], F32, tag="pstr", name="pstr")
                nc.tensor.transpose(pst[:], prs[:], id_sb[:])
                pbf = spool.tile([128, N_CC], BF16, tag="pbf", name="pbf")
                nc.vector.tensor_copy(pbf[:], pst[:])
                vt = val_tiles[b]
                ctx_sb = spool.tile([1, H], F32, tag="ctxsb", name="ctxsb")
                for half in range(2):
                    o0 = half * 512
                    ps = psctx.tile([1, 512], F32, tag="psctx", name="psctx")
                    for c in range(N_CC):
                        nc.tensor.matmul(
                            ps[:],
                            pbf[:, c:c + 1],
                            vt[:, c, o0:o0 + 512],
                            start=(c == 0), stop=(c == N_CC - 1))
                    nc.vector.tensor_scalar_mul(
                        ctx_sb[0:1, o0:o0 + 512], ps[:], rs[:])
                nc.scalar.dma_start(ctx_d[b:b + 1, :], ctx_sb[0:1, :])

            # ---- main emission loop (PE program order is emission order) ----
            from concourse.tile_rust import add_dep_helper

            N_PAIR = N_VC // 2
            load_key_quarter(0, 0)
            load_key_quarter(0, 1)
            pending = None
            for b in range(B_LOC):
                for jp in range(N_PAIR):
                    if jp == 0 and b > 0:
                        load_key_quarter(b, 2)
                        load_key_quarter(b, 3)
                        load_value(b)
                    if jp == 3 and b + 1 < B_LOC:
                        load_key_quarter(b + 1, 0)
                        load_key_quarter(b + 1, 1)
                    hts, last_mm = emit_kp_pair(b, jp)
                    if b == 0 and jp == 0:
                        # batch-0 bulk loads start only after the first kp
                        # pair's matmuls, so they don't steal DMA bandwidth
                        # from the startup-critical wk + keyT-quarter loads.
                        for d in (load_key_quarter(0, 2), load_key_quarter(0, 3),
                                  load_value(0)):
                            add_dep_helper(
                                d.ins, last_mm.ins, sync=True,
                                reason="defer batch-0 bulk loads past startup")
                    if pending is not None:
                        emit_score_pair(*pending)
                    pending = (b, jp, hts)
                    if jp == 0 and b > 0:
                        emit_softmax(b - 1)
                    if jp == 1 and b > 0:
                        emit_ctx(b - 1)
            emit_score_pair(*pending)
            emit_softmax(B_LOC - 1)
            emit_ctx(B_LOC - 1)

    nc.finalize()
    return nc


_NC_CACHE = None


def _get_nc():
    global _NC_CACHE
    if _NC_CACHE is None:
        _NC_CACHE = build_kernel()
    return _NC_CACHE


def prep_inputs(query, key, value, Wq, Wk, bias, Ws, bs):
    """Host-side shard + layout prep. Returns in_maps for the 8 cores."""
    query = np.asarray(query, dtype=np.float32)
    key = np.asarray(key, dtype=np.float32)
    value = np.asarray(value, dtype=np.float32)
    Wq = np.asarray(Wq, dtype=np.float32)
    Wk = np.asarray(Wk, dtype=np.float32)
    bias = np.asarray(bias, dtype=np.float32)
    Ws = np.asarray(Ws, dtype=np.float32)

    wkT = np.ascontiguousarray(Wk.T).astype(NP_BF16)            # [h, o]
    ws_p = np.ascontiguousarray(Ws.reshape(N_OC, 128).T).astype(NP_BF16)
    qb_full = (query[:, 0, :] @ Wq.T + bias).astype(np.float32)  # [B, H]
    ident = np.eye(32, dtype=np.float32)

    in_maps = []
    for c in range(N_CORES):
        sl = slice(c * B_LOC, (c + 1) * B_LOC)
        keyT = key[sl].transpose(0, 2, 1).astype(NP_BF16)        # [B_LOC, H, V]
        val_b = value[sl].astype(NP_BF16)                        # [B_LOC, V, H]
        qb = np.ascontiguousarray(
            qb_full[sl].reshape(B_LOC, N_OC, 128).transpose(2, 0, 1)
            .reshape(128, B_LOC * N_OC))
        in_maps.append({
            "keyT": np.ascontiguousarray(keyT),
            "value": np.ascontiguousarray(val_b),
            "wkT": wkT,
            "ws": ws_p,
            "qb": qb,
            "ident": ident,
        })
    return in_maps


def kernel(query, key, value, Wq, Wk, bias, Ws, bs):
    nc = _get_nc()
    in_maps = prep_inputs(query, key, value, Wq, Wk, bias, Ws, bs)
    res = run_bass_kernel_spmd(nc, in_maps, core_ids=list(range(N_CORES)))
    context = np.empty((B, Q, H), dtype=np.float32)
    attn = np.empty((B, V), dtype=np.float32)
    for c in range(N_CORES):
        sl = slice(c * B_LOC, (c + 1) * B_LOC)
        context[sl, 0, :] = res.results[c]["ctx_out"]
        attn[sl] = res.results[c]["attn_out"]
    return context, attn


# revision 22
# speedup vs baseline: 1.4109x; 1.2227x over previous
"""Additive (Bahdanau) attention kernel for Trainium2, SPMD over 8 NeuronCores.

Problem: B=32, Q=1, V=4096, H=1024 (fp32).
  kp    = key @ Wk^T                      (B, V, H)
  h     = tanh(kp + query @ Wq^T + bias)  (B, V, H)
  score = h @ Ws^T + bs                   (B, V)
  attn  = softmax(score)                  (B, V)   [bs drops: softmax shift-invariant]
  ctx   = attn @ value                    (B, 1, H)

Sharding: data-parallel over batch, 4 batches per core, no collectives.

Per-core dataflow (all matmuls bf16 with fp32 PSUM accumulation):
  - kp^T computed in PSUM blocks [o=128, v=512]: lhsT = WkT chunk, rhs = keyT
    (key is transposed to [H, V] on the host so the contraction dim (h) is the
    partition dim for both operands). Loop order (i, k, jj) shares each
    LDWEIGHTS across two 512-wide matmuls.
  - tanh fused on ScalarE with per-partition bias qb = query@Wq^T + bias
    (precomputed on host, (B,H) - tiny), output bf16 tiles h_t [o=128, v=512].
  - score[v] accumulated on PE: psum[1, 512] += Ws_chunk^T @ h_t over 8 o-chunks.
  - softmax on the [1, 4096] row: reduce_max (negated) -> Exp with bias=-max and
    fused accum_out sum -> reciprocal.
  - unnormalized p written contiguously to a DRAM scratch tile, read back as
    [32, 128], PE-transposed to [128, 32] (v on partitions), cast to bf16;
    ctx_raw[o] accumulated on PE: psum[1, o] += p_chunk^T @ value_chunk over
    32 v-chunks of 128 (value in natural [v, o] layout); final 1/sum scale is
    fused into the PSUM->SBUF copy.
"""

import numpy as np
import ml_dtypes

import concourse.bacc as bacc
import concourse.bass as bass
import concourse.mybir as mybir
import concourse.tile as tile
from concourse.bass_utils import run_bass_kernel_spmd

BF16 = mybir.dt.bfloat16
F32 = mybir.dt.float32
NP_BF16 = ml_dtypes.bfloat16

N_CORES = 8
B, Q, V, H = 32, 1, 4096, 1024
B_LOC = B // N_CORES          # 4 batches per core
VC = 512                      # v-chunk width for pass 1 (PSUM bank = 512 fp32)
N_VC = V // VC                # 8 v-chunks
N_OC = H // 128               # 8 o-chunks (output feature chunks)
N_HC = H // 128               # 8 h-chunks (contraction chunks)
N_CC = V // 128               # 32 v-chunks of 128 for the context matmul
QUARTER = V // 4              # keyT streamed in quarters of the v axis
PAIR = 2 * VC                 # v-chunk pair (one LDWEIGHTS serves both)


def build_kernel():
    nc = bacc.Bacc("TRN2", target_bir_lowering=False, debug=False,
                   num_devices=N_CORES)

    keyT_d = nc.declare_dram_parameter("keyT", [B_LOC, H, V], BF16, isOutput=False)
    val_d = nc.declare_dram_parameter("value", [B_LOC, V, H], BF16, isOutput=False)
    wkT_d = nc.declare_dram_parameter("wkT", [H, H], BF16, isOutput=False)
    ws_d = nc.declare_dram_parameter("ws", [128, N_OC], BF16, isOutput=False)
    qb_d = nc.declare_dram_parameter("qb", [128, B_LOC * N_OC], F32, isOutput=False)
    id_d = nc.declare_dram_parameter("ident", [32, 32], F32, isOutput=False)
    ctx_d = nc.declare_dram_parameter("ctx_out", [B_LOC, H], F32, isOutput=True)
    attn_d = nc.declare_dram_parameter("attn_out", [B_LOC, V], F32, isOutput=True)

    with tile.TileContext(nc) as tc:
        with (
            tc.tile_pool(name="weights", bufs=1) as wpool,
            tc.tile_pool(name="keyq", bufs=2) as kpool,
            tc.tile_pool(name="vals", bufs=1) as vpool,
            tc.tile_pool(name="ht", bufs=4 * N_OC) as htpool,
            tc.tile_pool(name="rows", bufs=1) as rowpool,
            tc.tile_pool(name="small", bufs=2) as spool,
            tc.tile_pool(name="stats", bufs=4) as stpool,
            tc.tile_pool(name="pdram", bufs=2, space="DRAM") as dpool,
            tc.tile_pool(name="pskp", bufs=4, space="PSUM") as pskp,
            tc.tile_pool(name="pssc", bufs=2, space="PSUM") as pssc,
            tc.tile_pool(name="pstr", bufs=1, space="PSUM") as pstr,
            tc.tile_pool(name="psctx", bufs=1, space="PSUM") as psctx,
        ):
            # ---- persistent weights (scalar-engine DMA queue, so the big
            # sync-queue keyT loads don't serialize behind them) ----
            wk_sb = wpool.tile([128, N_HC, H], BF16, tag="wk")
            nc.scalar.dma_start(
                wk_sb[:], wkT_d[:].rearrange("(k p) o -> p k o", p=128))
            ws_sb = wpool.tile([128, N_OC], BF16, tag="ws")
            nc.scalar.dma_start(ws_sb[:], ws_d[:])
            qb_sb = wpool.tile([128, B_LOC * N_OC], F32, tag="qb")
            nc.scalar.dma_start(qb_sb[:], qb_d[:])
            id_sb = wpool.tile([32, 32], F32, tag="ident")
            nc.scalar.dma_start(id_sb[:], id_d[:])

            key_tiles = {}   # (b, quarter) -> tile [128, N_HC, QUARTER]
            val_tiles = {}   # b -> tile [128, N_CC, H]

            def load_key_quarter(b, qtr):
                t = kpool.tile([128, N_HC, QUARTER], BF16, tag="keyq",
                               name="keyq")
                src = keyT_d[b].rearrange("(k p) v -> p k v", p=128)
                d = nc.sync.dma_start(
                    t[:], src[:, :, qtr * QUARTER:(qtr + 1) * QUARTER])
                key_tiles[(b, qtr)] = t
                return d

            def load_value(b):
                t = vpool.tile([128, N_CC, H], BF16, tag="val", name="val")
                d = nc.gpsimd.dma_start(
                    t[:], val_d[b].rearrange("(c p) o -> p c o", p=128))
                val_tiles[b] = t
                return d

            score_sb = {}    # b -> [1, V] f32

            def emit_kp_pair(b, jp):
                """Pair of v-chunks (2*jp, 2*jp+1): 128 matmuls + 16 tanh.

                Loop (i, k, jj): one LDWEIGHTS of wk[k, i-chunk] feeds both
                512-wide matmuls of the pair. Returns h_t tiles keyed (i, jj).
                """
                kt = key_tiles[(b, jp // (QUARTER // PAIR))]
                v0 = (jp % (QUARTER // PAIR)) * PAIR
                hts = {}
                first_mm = last_mm = None
                for i in range(N_OC):
                    pss = [pskp.tile([128, VC], F32, tag="pskp", name="pskp")
                           for _ in range(2)]
                    for k in range(N_HC):
                        for jj in range(2):
                            last_mm = nc.tensor.matmul(
                                pss[jj][:],
                                wk_sb[:, k, i * 128:(i + 1) * 128],
                                kt[:, k, v0 + jj * VC:v0 + (jj + 1) * VC],
                                start=(k == 0), stop=(k == N_HC - 1))
                            if first_mm is None:
                                first_mm = last_mm
                    for jj in range(2):
                        ht = htpool.tile([128, VC], BF16, tag="ht", name="ht")
                        nc.scalar.activation(
                            ht[:], pss[jj][:], mybir.ActivationFunctionType.Tanh,
                            bias=qb_sb[:, b * N_OC + i:b * N_OC + i + 1])
                        hts[(i, jj)] = ht
                return hts, first_mm, last_mm

            p_row = {}       # b -> [1, V] f32 unnormalized exp(score)
            p_sums = {}      # b -> [1, N_VC // 2] f32 per-pair exp sums
            pd_dram = {}     # b -> [32, 128] f32 DRAM scratch

            def emit_score_pair(b, jp, hts):
                """Score matmuls for v-chunk pair jp, then exp + DRAM scatter
                for the pair right away (overlapped with later kp pairs).

                Scores here are bounded (|score| <= sum|Ws| ~ 18), so exp in
                fp32 needs no max subtraction - softmax is shift-invariant and
                the reference's max-subtracted version is mathematically equal.
                """
                if b not in score_sb:
                    score_sb[b] = rowpool.tile([1, V], F32, tag="score",
                                               name="score", bufs=2)
                    p_row[b] = rowpool.tile([1, V], F32, tag="p", name="p",
                                            bufs=1)
                    p_sums[b] = stpool.tile([1, N_VC // 2], F32, tag="psums",
                                            name="psums")
                    pd_dram[b] = dpool.tile([32, 128], F32, tag="pd", name="pd")
                for jj in range(2):
                    j = 2 * jp + jj
                    ps = pssc.tile([1, VC], F32, tag="pssc", name="pssc")
                    for i in range(N_OC):
                        nc.tensor.matmul(
                            ps[:], ws_sb[:, i:i + 1], hts[(i, jj)][:],
                            start=(i == 0), stop=(i == N_OC - 1))
                    nc.vector.tensor_copy(
                        score_sb[b][0:1, j * VC:(j + 1) * VC], ps[:])
                seg = slice(jp * PAIR, (jp + 1) * PAIR)
                nc.scalar.activation(
                    p_row[b][0:1, seg], score_sb[b][0:1, seg],
                    mybir.ActivationFunctionType.Exp,
                    accum_out=p_sums[b][0:1, jp:jp + 1])
                nc.scalar.dma_start(
                    pd_dram[b][8 * jp:8 * (jp + 1), :]
                    .rearrange("c p -> (c p)")[None, :],
                    p_row[b][0:1, seg])

            p_back = {}      # b -> ([32, 128] f32 tile, rs [1,1] tile)

            def emit_softmax(b):
                sm = stpool.tile([1, 1], F32, tag="sm", name="sm")
                nc.vector.reduce_sum(
                    sm[:], p_sums[b][:], axis=mybir.AxisListType.X)
                rs = stpool.tile([1, 1], F32, tag="rs", name="rs")
                nc.vector.reciprocal(rs[:], sm[:])
                # read the scattered p back as [32, 128] (v = c*128 + p)
                prs = spool.tile([32, 128], F32, tag="prs", name="prs")
                nc.scalar.dma_start(prs[:], pd_dram[b][:])
                p_back[b] = (prs, rs)
                # normalized attn output
                attn_f = rowpool.tile([1, V], F32, tag="score", name="attnf",
                                      bufs=2)
                nc.vector.tensor_scalar_mul(attn_f[:], p_row[b][:], rs[:])
                nc.scalar.dma_start(attn_d[b:b + 1, :], attn_f[0:1, :])

            def emit_ctx(b):
                prs, rs = p_back[b]
                # transpose [32, 128] -> psum [128, 32] on PE
                pst = pstr.tile([128, 32], F32, tag="pstr", name="pstr")
                nc.tensor.transpose(pst[:], prs[:], id_sb[:])
                pbf = spool.tile([128, N_CC], BF16, tag="pbf", name="pbf")
                nc.vector.tensor_copy(pbf[:], pst[:])
                vt = val_tiles[b]
                ctx_sb = spool.tile([1, H], F32, tag="ctxsb", name="ctxsb")
                for half in range(2):
                    o0 = half * 512
                    ps = psctx.tile([1, 512], F32, tag="psctx", name="psctx")
                    for c in range(N_CC):
                        nc.tensor.matmul(
                            ps[:],
                            pbf[:, c:c + 1],
                            vt[:, c, o0:o0 + 512],
                            start=(c == 0), stop=(c == N_CC - 1))
                    nc.vector.tensor_scalar_mul(
                        ctx_sb[0:1, o0:o0 + 512], ps[:], rs[:])
                nc.scalar.dma_start(ctx_d[b:b + 1, :], ctx_sb[0:1, :])

            # ---- main emission loop (PE program order is emission order) ----
            from concourse.tile_rust import add_dep_helper

            N_PAIR = N_VC // 2
            load_key_quarter(0, 0)
            pending = None
            for b in range(B_LOC):
                for jp in range(N_PAIR):
                    if jp == 0 and b > 0:
                        load_key_quarter(b, 2)
                        load_key_quarter(b, 3)
                        load_value(b)
                    if jp == 3 and b + 1 < B_LOC:
                        load_key_quarter(b + 1, 0)
                        load_key_quarter(b + 1, 1)
                    hts, first_mm, last_mm = emit_kp_pair(b, jp)
                    if b == 0 and jp == 0:
                        # batch-0 bulk loads start only after the first kp
                        # matmul and run chained, so each gets full DMA
                        # bandwidth in the order the compute needs it.
                        prev = first_mm
                        for d in (load_key_quarter(0, 1), load_key_quarter(0, 2),
                                  load_key_quarter(0, 3), load_value(0)):
                            add_dep_helper(
                                d.ins, prev.ins, sync=True,
                                reason="chain batch-0 bulk loads past startup")
                            prev = d
                    if pending is not None:
                        emit_score_pair(*pending)
                    pending = (b, jp, hts)
                    if jp == 0 and b > 0:
                        emit_softmax(b - 1)
                    if jp == 1 and b > 0:
                        emit_ctx(b - 1)
            emit_score_pair(*pending)
            emit_softmax(B_LOC - 1)
            emit_ctx(B_LOC - 1)

    nc.finalize()
    return nc


_NC_CACHE = None


def _get_nc():
    global _NC_CACHE
    if _NC_CACHE is None:
        _NC_CACHE = build_kernel()
    return _NC_CACHE


def prep_inputs(query, key, value, Wq, Wk, bias, Ws, bs):
    """Host-side shard + layout prep. Returns in_maps for the 8 cores."""
    query = np.asarray(query, dtype=np.float32)
    key = np.asarray(key, dtype=np.float32)
    value = np.asarray(value, dtype=np.float32)
    Wq = np.asarray(Wq, dtype=np.float32)
    Wk = np.asarray(Wk, dtype=np.float32)
    bias = np.asarray(bias, dtype=np.float32)
    Ws = np.asarray(Ws, dtype=np.float32)

    wkT = np.ascontiguousarray(Wk.T).astype(NP_BF16)            # [h, o]
    ws_p = np.ascontiguousarray(Ws.reshape(N_OC, 128).T).astype(NP_BF16)
    qb_full = (query[:, 0, :] @ Wq.T + bias).astype(np.float32)  # [B, H]
    ident = np.eye(32, dtype=np.float32)

    in_maps = []
    for c in range(N_CORES):
        sl = slice(c * B_LOC, (c + 1) * B_LOC)
        keyT = key[sl].transpose(0, 2, 1).astype(NP_BF16)        # [B_LOC, H, V]
        val_b = value[sl].astype(NP_BF16)                        # [B_LOC, V, H]
        qb = np.ascontiguousarray(
            qb_full[sl].reshape(B_LOC, N_OC, 128).transpose(2, 0, 1)
            .reshape(128, B_LOC * N_OC))
        in_maps.append({
            "keyT": np.ascontiguousarray(keyT),
            "value": np.ascontiguousarray(val_b),
            "wkT": wkT,
            "ws": ws_p,
            "qb": qb,
            "ident": ident,
        })
    return in_maps


def kernel(query, key, value, Wq, Wk, bias, Ws, bs):
    nc = _get_nc()
    in_maps = prep_inputs(query, key, value, Wq, Wk, bias, Ws, bs)
    res = run_bass_kernel_spmd(nc, in_maps, core_ids=list(range(N_CORES)))
    context = np.empty((B, Q, H), dtype=np.float32)
    attn = np.empty((B, V), dtype=np.float32)
    for c in range(N_CORES):
        sl = slice(c * B_LOC, (c + 1) * B_LOC)
        context[sl, 0, :] = res.results[c]["ctx_out"]
        attn[sl] = res.results[c]["attn_out"]
    return context, attn


# revision 26
# speedup vs baseline: 1.4181x; 1.0051x over previous
"""Additive (Bahdanau) attention kernel for Trainium2, SPMD over 8 NeuronCores.

Problem: B=32, Q=1, V=4096, H=1024 (fp32).
  kp    = key @ Wk^T                      (B, V, H)
  h     = tanh(kp + query @ Wq^T + bias)  (B, V, H)
  score = h @ Ws^T + bs                   (B, V)
  attn  = softmax(score)                  (B, V)   [bs drops: softmax shift-invariant]
  ctx   = attn @ value                    (B, 1, H)

Sharding: data-parallel over batch, 4 batches per core, no collectives.

Per-core dataflow (all matmuls bf16 with fp32 PSUM accumulation):
  - kp^T computed in PSUM blocks [o=128, v=512]: lhsT = WkT chunk, rhs = keyT
    (key is transposed to [H, V] on the host so the contraction dim (h) is the
    partition dim for both operands). Loop order (i, k, jj) shares each
    LDWEIGHTS across two 512-wide matmuls.
  - tanh fused on ScalarE with per-partition bias qb = query@Wq^T + bias
    (precomputed on host, (B,H) - tiny), output bf16 tiles h_t [o=128, v=512].
  - score[v] accumulated on PE: psum[1, 512] += Ws_chunk^T @ h_t over 8 o-chunks.
  - softmax on the [1, 4096] row: reduce_max (negated) -> Exp with bias=-max and
    fused accum_out sum -> reciprocal.
  - unnormalized p written contiguously to a DRAM scratch tile, read back as
    [32, 128], PE-transposed to [128, 32] (v on partitions), cast to bf16;
    ctx_raw[o] accumulated on PE: psum[1, o] += p_chunk^T @ value_chunk over
    32 v-chunks of 128 (value in natural [v, o] layout); final 1/sum scale is
    fused into the PSUM->SBUF copy.
"""

import numpy as np
import ml_dtypes

import concourse.bacc as bacc
import concourse.bass as bass
import concourse.mybir as mybir
import concourse.tile as tile
from concourse.bass_utils import run_bass_kernel_spmd

BF16 = mybir.dt.bfloat16
F32 = mybir.dt.float32
NP_BF16 = ml_dtypes.bfloat16

N_CORES = 8
B, Q, V, H = 32, 1, 4096, 1024
B_LOC = B // N_CORES          # 4 batches per core
VC = 512                      # v-chunk width for pass 1 (PSUM bank = 512 fp32)
N_VC = V // VC                # 8 v-chunks
N_OC = H // 128               # 8 o-chunks (output feature chunks)
N_HC = H // 128               # 8 h-chunks (contraction chunks)
N_CC = V // 128               # 32 v-chunks of 128 for the context matmul
QUARTER = V // 4              # keyT streamed in quarters of the v axis
PAIR = 2 * VC                 # v-chunk pair (one LDWEIGHTS serves both)


def build_kernel():
    nc = bacc.Bacc("TRN2", target_bir_lowering=False, debug=False,
                   num_devices=N_CORES)

    # pre-tiled host layouts: partition lines are fully contiguous in DRAM
    keyT_d = nc.declare_dram_parameter(
        "keyT", [B_LOC, 4, 128, N_HC, QUARTER], BF16, isOutput=False)
    val_d = nc.declare_dram_parameter(
        "value", [B_LOC, 128, N_CC, H], BF16, isOutput=False)
    wkT_d = nc.declare_dram_parameter(
        "wkT", [128, N_HC, H], BF16, isOutput=False)
    ws_d = nc.declare_dram_parameter("ws", [128, N_OC], BF16, isOutput=False)
    qb_d = nc.declare_dram_parameter("qb", [128, B_LOC * N_OC], F32, isOutput=False)
    id_d = nc.declare_dram_parameter("ident", [32, 32], F32, isOutput=False)
    ctx_d = nc.declare_dram_parameter("ctx_out", [B_LOC, H], F32, isOutput=True)
    attn_d = nc.declare_dram_parameter("attn_out", [B_LOC, V], F32, isOutput=True)

    with tile.TileContext(nc) as tc:
        with (
            tc.tile_pool(name="weights", bufs=1) as wpool,
            tc.tile_pool(name="keyq", bufs=2) as kpool,
            tc.tile_pool(name="vals", bufs=1) as vpool,
            tc.tile_pool(name="ht", bufs=4 * N_OC) as htpool,
            tc.tile_pool(name="rows", bufs=1) as rowpool,
            tc.tile_pool(name="small", bufs=2) as spool,
            tc.tile_pool(name="stats", bufs=4) as stpool,
            tc.tile_pool(name="pdram", bufs=2, space="DRAM") as dpool,
            tc.tile_pool(name="pskp", bufs=4, space="PSUM") as pskp,
            tc.tile_pool(name="pssc", bufs=2, space="PSUM") as pssc,
            tc.tile_pool(name="pstr", bufs=1, space="PSUM") as pstr,
            tc.tile_pool(name="psctx", bufs=1, space="PSUM") as psctx,
        ):
            # ---- persistent weights. wk on the gpsimd queue: the scalar
            # queue is blocked by ACT_TABLE_LOAD at startup and the sync
            # queue carries the startup-critical first keyT quarter. ----
            wk_sb = wpool.tile([128, N_HC, H], BF16, tag="wk")
            nc.gpsimd.dma_start(wk_sb[:], wkT_d[:])
            ws_sb = wpool.tile([128, N_OC], BF16, tag="ws")
            nc.scalar.dma_start(ws_sb[:], ws_d[:])
            qb_sb = wpool.tile([128, B_LOC * N_OC], F32, tag="qb")
            nc.scalar.dma_start(qb_sb[:], qb_d[:])
            id_sb = wpool.tile([32, 32], F32, tag="ident")
            nc.scalar.dma_start(id_sb[:], id_d[:])

            key_tiles = {}   # (b, quarter) -> tile [128, N_HC, QUARTER]
            val_tiles = {}   # b -> tile [128, N_CC, H]

            def load_key_quarter(b, qtr):
                t = kpool.tile([128, N_HC, QUARTER], BF16, tag="keyq",
                               name="keyq")
                d = nc.sync.dma_start(t[:], keyT_d[b, qtr])
                key_tiles[(b, qtr)] = t
                return d

            def load_value(b):
                t = vpool.tile([128, N_CC, H], BF16, tag="val", name="val")
                d = nc.gpsimd.dma_start(t[:], val_d[b])
                val_tiles[b] = t
                return d

            score_sb = {}    # b -> [1, V] f32

            def emit_kp_pair(b, jp):
                """Pair of v-chunks (2*jp, 2*jp+1): 128 matmuls + 16 tanh.

                Loop (i, k, jj): one LDWEIGHTS of wk[k, i-chunk] feeds both
                512-wide matmuls of the pair. Returns h_t tiles keyed (i, jj).
                """
                kt = key_tiles[(b, jp // (QUARTER // PAIR))]
                v0 = (jp % (QUARTER // PAIR)) * PAIR
                hts = {}
                first_mm = last_mm = None
                for i in range(N_OC):
                    pss = [pskp.tile([128, VC], F32, tag="pskp", name="pskp")
                           for _ in range(2)]
                    for k in range(N_HC):
                        for jj in range(2):
                            last_mm = nc.tensor.matmul(
                                pss[jj][:],
                                wk_sb[:, k, i * 128:(i + 1) * 128],
                                kt[:, k, v0 + jj * VC:v0 + (jj + 1) * VC],
                                start=(k == 0), stop=(k == N_HC - 1))
                            if first_mm is None:
                                first_mm = last_mm
                    for jj in range(2):
                        ht = htpool.tile([128, VC], BF16, tag="ht", name="ht")
                        nc.scalar.activation(
                            ht[:], pss[jj][:], mybir.ActivationFunctionType.Tanh,
                            bias=qb_sb[:, b * N_OC + i:b * N_OC + i + 1])
                        hts[(i, jj)] = ht
                return hts, first_mm, last_mm

            p_row = {}       # b -> [1, V] f32 unnormalized exp(score)
            p_sums = {}      # b -> [1, N_VC // 2] f32 per-pair exp sums
            pd_dram = {}     # b -> [32, 128] f32 DRAM scratch

            def emit_score_pair(b, jp, hts):
                """Score matmuls for v-chunk pair jp, then exp + DRAM scatter
                for the pair right away (overlapped with later kp pairs).

                Scores here are bounded (|score| <= sum|Ws| ~ 18), so exp in
                fp32 needs no max subtraction - softmax is shift-invariant and
                the reference's max-subtracted version is mathematically equal.
                """
                if b not in score_sb:
                    score_sb[b] = rowpool.tile([1, V], F32, tag="score",
                                               name="score", bufs=2)
                    p_row[b] = rowpool.tile([1, V], F32, tag="p", name="p",
                                            bufs=1)
                    p_sums[b] = stpool.tile([1, N_VC // 2], F32, tag="psums",
                                            name="psums")
                    pd_dram[b] = dpool.tile([32, 128], F32, tag="pd", name="pd")
                for jj in range(2):
                    j = 2 * jp + jj
                    ps = pssc.tile([1, VC], F32, tag="pssc", name="pssc")
                    for i in range(N_OC):
                        nc.tensor.matmul(
                            ps[:], ws_sb[:, i:i + 1], hts[(i, jj)][:],
                            start=(i == 0), stop=(i == N_OC - 1))
                    nc.vector.tensor_copy(
                        score_sb[b][0:1, j * VC:(j + 1) * VC], ps[:])
                seg = slice(jp * PAIR, (jp + 1) * PAIR)
                nc.scalar.activation(
                    p_row[b][0:1, seg], score_sb[b][0:1, seg],
                    mybir.ActivationFunctionType.Exp,
                    accum_out=p_sums[b][0:1, jp:jp + 1])
                nc.scalar.dma_start(
                    pd_dram[b][8 * jp:8 * (jp + 1), :]
                    .rearrange("c p -> (c p)")[None, :],
                    p_row[b][0:1, seg])

            p_back = {}      # b -> ([32, 128] f32 tile, rs [1,1] tile)

            def emit_softmax(b):
                sm = stpool.tile([1, 1], F32, tag="sm", name="sm")
                nc.vector.reduce_sum(
                    sm[:], p_sums[b][:], axis=mybir.AxisListType.X)
                rs = stpool.tile([1, 1], F32, tag="rs", name="rs")
                nc.vector.reciprocal(rs[:], sm[:])
                # read the scattered p back as [32, 128] (v = c*128 + p)
                prs = spool.tile([32, 128], F32, tag="prs", name="prs")
                nc.scalar.dma_start(prs[:], pd_dram[b][:])
                p_back[b] = (prs, rs)
                # normalized attn output
                attn_f = rowpool.tile([1, V], F32, tag="score", name="attnf",
                                      bufs=2)
                nc.vector.tensor_scalar_mul(attn_f[:], p_row[b][:], rs[:])
                nc.scalar.dma_start(attn_d[b:b + 1, :], attn_f[0:1, :])

            def emit_ctx(b):
                prs, rs = p_back[b]
                # transpose [32, 128] -> psum [128, 32] on PE
                pst = pstr.tile([128, 32], F32, tag="pstr", name="pstr")
                nc.tensor.transpose(pst[:], prs[:], id_sb[:])
                pbf = spool.tile([128, N_CC], BF16, tag="pbf", name="pbf")
                nc.vector.tensor_copy(pbf[:], pst[:])
                vt = val_tiles[b]
                ctx_sb = spool.tile([1, H], F32, tag="ctxsb", name="ctxsb")
                for half in range(2):
                    o0 = half * 512
                    ps = psctx.tile([1, 512], F32, tag="psctx", name="psctx")
                    for c in range(N_CC):
                        nc.tensor.matmul(
                            ps[:],
                            pbf[:, c:c + 1],
                            vt[:, c, o0:o0 + 512],
                            start=(c == 0), stop=(c == N_CC - 1))
                    nc.vector.tensor_scalar_mul(
                        ctx_sb[0:1, o0:o0 + 512], ps[:], rs[:])
                nc.scalar.dma_start(ctx_d[b:b + 1, :], ctx_sb[0:1, :])

            # ---- main emission loop (PE program order is emission order) ----
            from concourse.tile_rust import add_dep_helper

            N_PAIR = N_VC // 2
            load_key_quarter(0, 0)
            pending = None
            for b in range(B_LOC):
                for jp in range(N_PAIR):
                    if jp == 0 and b > 0:
                        load_key_quarter(b, 2)
                        load_key_quarter(b, 3)
                        load_value(b)
                    if jp == 3 and b + 1 < B_LOC:
                        load_key_quarter(b + 1, 0)
                        load_key_quarter(b + 1, 1)
                    hts, first_mm, last_mm = emit_kp_pair(b, jp)
                    if b == 0 and jp == 0:
                        # batch-0 bulk loads start only after the first kp
                        # matmul and run chained, so each gets full DMA
                        # bandwidth in the order the compute needs it.
                        prev = first_mm
                        for d in (load_key_quarter(0, 1), load_key_quarter(0, 2),
                                  load_key_quarter(0, 3), load_value(0)):
                            add_dep_helper(
                                d.ins, prev.ins, sync=True,
                                reason="chain batch-0 bulk loads past startup")
                            prev = d
                    if pending is not None:
                        emit_score_pair(*pending)
                    pending = (b, jp, hts)
                    if jp == 0 and b > 0:
                        emit_softmax(b - 1)
                    if jp == 1 and b > 0:
                        emit_ctx(b - 1)
            emit_score_pair(*pending)
            emit_softmax(B_LOC - 1)
            emit_ctx(B_LOC - 1)

    nc.finalize()
    return nc


_NC_CACHE = None


def _get_nc():
    global _NC_CACHE
    if _NC_CACHE is None:
        _NC_CACHE = build_kernel()
    return _NC_CACHE


def prep_inputs(query, key, value, Wq, Wk, bias, Ws, bs):
    """Host-side shard + layout prep. Returns in_maps for the 8 cores."""
    query = np.asarray(query, dtype=np.float32)
    key = np.asarray(key, dtype=np.float32)
    value = np.asarray(value, dtype=np.float32)
    Wq = np.asarray(Wq, dtype=np.float32)
    Wk = np.asarray(Wk, dtype=np.float32)
    bias = np.asarray(bias, dtype=np.float32)
    Ws = np.asarray(Ws, dtype=np.float32)

    # wkT[p, k, o] = Wk[o, k*128+p]
    wkT = np.ascontiguousarray(
        Wk.T.reshape(N_HC, 128, H).transpose(1, 0, 2)).astype(NP_BF16)
    ws_p = np.ascontiguousarray(Ws.reshape(N_OC, 128).T).astype(NP_BF16)
    qb_full = (query[:, 0, :] @ Wq.T + bias).astype(np.float32)  # [B, H]
    ident = np.eye(32, dtype=np.float32)

    in_maps = []
    for c in range(N_CORES):
        sl = slice(c * B_LOC, (c + 1) * B_LOC)
        # keyT[b, q, p, k, vv] = key[b, q*1024+vv, k*128+p]
        keyT = (key[sl].reshape(B_LOC, 4, QUARTER, N_HC, 128)
                .transpose(0, 1, 4, 3, 2).astype(NP_BF16))
        # val[b, p, c, o] = value[b, c*128+p, o]
        val_b = (value[sl].reshape(B_LOC, N_CC, 128, H)
                 .transpose(0, 2, 1, 3).astype(NP_BF16))
        qb = np.ascontiguousarray(
            qb_full[sl].reshape(B_LOC, N_OC, 128).transpose(2, 0, 1)
            .reshape(128, B_LOC * N_OC))
        in_maps.append({
            "keyT": np.ascontiguousarray(keyT),
            "value": np.ascontiguousarray(val_b),
            "wkT": wkT,
            "ws": ws_p,
            "qb": qb,
            "ident": ident,
        })
    return in_maps


def kernel(query, key, value, Wq, Wk, bias, Ws, bs):
    nc = _get_nc()
    in_maps = prep_inputs(query, key, value, Wq, Wk, bias, Ws, bs)
    res = run_bass_kernel_spmd(nc, in_maps, core_ids=list(range(N_CORES)))
    context = np.empty((B, Q, H), dtype=np.float32)
    attn = np.empty((B, V), dtype=np.float32)
    for c in range(N_CORES):
        sl = slice(c * B_LOC, (c + 1) * B_LOC)
        context[sl, 0, :] = res.results[c]["ctx_out"]
        attn[sl] = res.results[c]["attn_out"]
    return context, attn


# revision 36
# speedup vs baseline: 1.4195x; 1.0010x over previous
"""Additive (Bahdanau) attention kernel for Trainium2, SPMD over 8 NeuronCores.

Problem: B=32, Q=1, V=4096, H=1024 (fp32).
  kp    = key @ Wk^T                      (B, V, H)
  h     = tanh(kp + query @ Wq^T + bias)  (B, V, H)
  score = h @ Ws^T + bs                   (B, V)
  attn  = softmax(score)                  (B, V)   [bs drops: softmax shift-invariant]
  ctx   = attn @ value                    (B, 1, H)

Sharding: data-parallel over batch, 4 batches per core, no collectives.

Per-core dataflow (matmuls bf16 with fp32 PSUM accumulation), pipelined over
v-chunk pairs (1024 keys each), with each stage deferred in the PE stream so
the TensorEngine never waits on the softmax chain:
  stage s:   kp^T pair matmuls -> tanh (ScalarE, fused qb bias) -> h_t bf16
  stage s-1: score matmuls (Ws^T @ h_t accumulated over o-chunks) -> exp of the
             pair (no max subtraction: scores are bounded, fp32 exp is exact
             enough and softmax is shift-invariant) with fused pair-sum ->
             contiguous DRAM scatter of p -> strided read-back [8, 128]
  stage s-2: PE transpose [8,128] -> [128,8] (v on partitions), cast bf16,
             16 context matmuls (p_chunk^T @ value_chunk, accumulated in PSUM
             across the batch)
  batch end: sum of pair-sums -> reciprocal -> attn = p * rs (DMA out);
             ctx = psum * rs (DMA out).
"""

import numpy as np
import ml_dtypes

import concourse.bacc as bacc
import concourse.bass as bass
import concourse.mybir as mybir
import concourse.tile as tile
from concourse.bass_utils import run_bass_kernel_spmd

BF16 = mybir.dt.bfloat16
F32 = mybir.dt.float32
NP_BF16 = ml_dtypes.bfloat16

N_CORES = 8
B, Q, V, H = 32, 1, 4096, 1024
B_LOC = B // N_CORES          # 4 batches per core
VC = 512                      # v-chunk width for pass 1 (PSUM bank = 512 fp32)
N_VC = V // VC                # 8 v-chunks
N_OC = H // 128               # 8 o-chunks (output feature chunks)
N_HC = H // 128               # 8 h-chunks (contraction chunks)
N_CC = V // 128               # 32 v-chunks of 128 for the context matmul
PAIR = 2 * VC                 # v-chunk pair; also the keyT streaming quantum
N_PAIR = V // PAIR            # 4 pairs per batch


def build_kernel():
    nc = bacc.Bacc("TRN2", target_bir_lowering=False, debug=False,
                   num_devices=N_CORES)

    # pre-tiled host layouts: partition lines are fully contiguous in DRAM
    keyT_d = nc.declare_dram_parameter(
        "keyT", [B_LOC, N_PAIR, 128, N_HC, PAIR], BF16, isOutput=False)
    val_d = nc.declare_dram_parameter(
        "value", [B_LOC, 128, N_CC, H], BF16, isOutput=False)
    wkT_d = nc.declare_dram_parameter(
        "wkT", [128, N_HC, H], BF16, isOutput=False)
    ws_d = nc.declare_dram_parameter("ws", [128, N_OC], BF16, isOutput=False)
    qb_d = nc.declare_dram_parameter("qb", [128, B_LOC * N_OC], F32, isOutput=False)
    id_d = nc.declare_dram_parameter("ident", [32, 32], F32, isOutput=False)
    ctx_d = nc.declare_dram_parameter("ctx_out", [B_LOC, H], F32, isOutput=True)
    attn_d = nc.declare_dram_parameter("attn_out", [B_LOC, V], F32, isOutput=True)

    with tile.TileContext(nc) as tc:
        with (
            tc.tile_pool(name="weights", bufs=1) as wpool,
            tc.tile_pool(name="keyq", bufs=2) as kpool,
            tc.tile_pool(name="vals", bufs=1) as vpool,
            tc.tile_pool(name="ht", bufs=20) as htpool,
            tc.tile_pool(name="rows", bufs=1) as rowpool,
            tc.tile_pool(name="small", bufs=2) as spool,
            tc.tile_pool(name="stats", bufs=4) as stpool,
            tc.tile_pool(name="pdram", bufs=2, space="DRAM") as dpool,
            tc.tile_pool(name="pskp", bufs=3, space="PSUM") as pskp,
            tc.tile_pool(name="pssc", bufs=2, space="PSUM") as pssc,
            tc.tile_pool(name="trctx", bufs=3, space="PSUM") as trctx,
        ):
            # ---- persistent weights. wk on the gpsimd queue (the scalar
            # queue is blocked by ACT_TABLE_LOAD at startup, the sync queue
            # carries the startup-critical first keyT pair); split per
            # o-chunk so the first kp matmuls only wait for 256KB. ----
            wk_sb = wpool.tile([128, N_HC, H], BF16, tag="wk")
            for i in range(N_OC):
                nc.gpsimd.dma_start(wk_sb[:, :, i * 128:(i + 1) * 128],
                                    wkT_d[:, :, i * 128:(i + 1) * 128])
            ws_sb = wpool.tile([128, N_OC], BF16, tag="ws")
            nc.scalar.dma_start(ws_sb[:], ws_d[:])
            qb_sb = wpool.tile([128, B_LOC * N_OC], F32, tag="qb")
            nc.scalar.dma_start(qb_sb[:], qb_d[:])
            id_sb = wpool.tile([32, 32], F32, tag="ident")
            nc.scalar.dma_start(id_sb[:], id_d[:])

            key_tiles = {}   # (b, jp) -> tile [128, N_HC, PAIR]
            val_tiles = {}   # b -> tile [128, N_CC, H]

            def load_key_pair(b, jp):
                t = kpool.tile([128, N_HC, PAIR], BF16, tag="keyq",
                               name="keyq", bufs=3)
                d = nc.sync.dma_start(t[:], keyT_d[b, jp])
                key_tiles[(b, jp)] = t
                return d

            def load_value(b):
                t = vpool.tile([128, N_CC, H], BF16, tag="val", name="val")
                d = nc.gpsimd.dma_start(t[:], val_d[b])
                val_tiles[b] = t
                return d

            # ---- per-batch state ----
            score_sb = {}    # b -> [1, V] f32
            p_row = {}       # b -> [1, V] f32 unnormalized exp(score)
            p_sums = {}      # b -> [1, N_PAIR] f32 per-pair exp sums
            pd_dram = {}     # b -> [32, 128] f32 DRAM scratch (v = c*128 + p)
            prs_sb = {}      # (b, jp) -> [8, 128] f32 read-back (partition 0)
            pbf_sb = {}      # b -> [128, N_CC] bf16 (v on partitions)
            ctx_ps = {}      # b -> two [1, 512] psum halves
            rs_st = {}       # b -> [1, 1] f32 reciprocal of softmax sum

            def batch_state(b):
                if b in score_sb:
                    return
                score_sb[b] = rowpool.tile([1, V], F32, tag="score",
                                           name="score", bufs=2)
                p_row[b] = rowpool.tile([1, V], F32, tag="p", name="p", bufs=1)
                p_sums[b] = stpool.tile([1, N_PAIR], F32, tag="psums",
                                        name="psums")
                pd_dram[b] = dpool.tile([32, 128], F32, tag="pd", name="pd")
                pbf_sb[b] = spool.tile([128, N_CC], BF16, tag="pbf", name="pbf")

            def emit_kp_pair(b, jp):
                """Pair of v-chunks: 128 matmuls + 16 tanh -> h_t (i, jj)."""
                kt = key_tiles[(b, jp)]
                hts = {}
                first_mm = last_mm = None
                for i in range(N_OC):
                    pss = [pskp.tile([128, VC], F32, tag="pskp", name="pskp")
                           for _ in range(2)]
                    for k in range(N_HC):
                        for jj in range(2):
                            last_mm = nc.tensor.matmul(
                                pss[jj][:],
                                wk_sb[:, k, i * 128:(i + 1) * 128],
                                kt[:, k, jj * VC:(jj + 1) * VC],
                                start=(k == 0), stop=(k == N_HC - 1))
                            if first_mm is None:
                                first_mm = last_mm
                    for jj in range(2):
                        ht = htpool.tile([128, VC], BF16, tag="ht", name="ht")
                        nc.scalar.activation(
                            ht[:], pss[jj][:], mybir.ActivationFunctionType.Tanh,
                            bias=qb_sb[:, b * N_OC + i:b * N_OC + i + 1])
                        hts[(i, jj)] = ht
                return hts, first_mm, last_mm

            def emit_score_pair(b, jp, hts):
                """Score matmuls, then exp + pair-sum + DRAM scatter + strided
                read-back for the pair (all overlapped with later kp pairs)."""
                batch_state(b)
                for jj in range(2):
                    j = 2 * jp + jj
                    ps = pssc.tile([1, VC], F32, tag="pssc", name="pssc")
                    for i in range(N_OC):
                        nc.tensor.matmul(
                            ps[:], ws_sb[:, i:i + 1], hts[(i, jj)][:],
                            start=(i == 0), stop=(i == N_OC - 1))
                    nc.vector.tensor_copy(
                        score_sb[b][0:1, j * VC:(j + 1) * VC], ps[:])
                seg = slice(jp * PAIR, (jp + 1) * PAIR)
                rows = slice(8 * jp, 8 * (jp + 1))
                nc.scalar.activation(
                    p_row[b][0:1, seg], score_sb[b][0:1, seg],
                    mybir.ActivationFunctionType.Exp,
                    accum_out=p_sums[b][0:1, jp:jp + 1])
                nc.scalar.dma_start(
                    pd_dram[b][rows, :].rearrange("c p -> (c p)")[None, :],
                    p_row[b][0:1, seg])
                prs = spool.tile([8, 128], F32, tag="prs", name="prs", bufs=3)
                nc.scalar.dma_start(prs[:], pd_dram[b][rows, :])
                prs_sb[(b, jp)] = prs

            def emit_ctx_pair(b, jp):
                """PE transpose of the pair's p to [128, 8], cast to bf16,
                16 context matmuls accumulating into the batch psum halves."""
                rows = slice(8 * jp, 8 * (jp + 1))
                pst = trctx.tile([128, 8], F32, tag="pstr", name="pstr",
                                 bufs=1)
                nc.tensor.transpose(pst[:], prs_sb[(b, jp)][:],
                                    id_sb[0:8, 0:8])
                nc.vector.tensor_copy(pbf_sb[b][:, rows], pst[:])
                if b not in ctx_ps:
                    ctx_ps[b] = [trctx.tile([1, 512], F32, tag="psctx",
                                            name="psctx", bufs=2)
                                 for _ in range(2)]
                vt = val_tiles[b]
                for half in range(2):
                    o0 = half * 512
                    for cc in range(8):
                        c = jp * 8 + cc
                        nc.tensor.matmul(
                            ctx_ps[b][half][:],
                            pbf_sb[b][:, c:c + 1],
                            vt[:, c, o0:o0 + 512],
                            start=(jp == 0 and cc == 0),
                            stop=(jp == N_PAIR - 1 and cc == 7))

            def emit_softmax(b):
                sm = stpool.tile([1, 1], F32, tag="sm", name="sm")
                nc.vector.reduce_sum(
                    sm[:], p_sums[b][:], axis=mybir.AxisListType.X)
                rs = stpool.tile([1, 1], F32, tag="rs", name="rs")
                nc.vector.reciprocal(rs[:], sm[:])
                rs_st[b] = rs
                attn_f = rowpool.tile([1, V], F32, tag="score", name="attnf",
                                      bufs=2)
                nc.vector.tensor_scalar_mul(attn_f[:], p_row[b][:], rs[:])
                nc.scalar.dma_start(attn_d[b:b + 1, :], attn_f[0:1, :])

            def emit_ctx_finish(b):
                ctx_sb = spool.tile([1, H], F32, tag="ctxsb", name="ctxsb", bufs=1)
                for half in range(2):
                    nc.vector.tensor_scalar_mul(
                        ctx_sb[0:1, half * 512:(half + 1) * 512],
                        ctx_ps[b][half][:], rs_st[b][:])
                nc.scalar.dma_start(ctx_d[b:b + 1, :], ctx_sb[0:1, :])

            # ---- main emission loop; PE program order is emission order.
            # Stages are deferred so the PE stream never waits on the
            # softmax/reshape chain of the same pair. ----
            from concourse.tile_rust import add_dep_helper

            load_key_pair(0, 0)
            sc_q = []    # pending (b, jp, hts) for score stage (defer 1)
            tr_q = []    # pending (b, jp) for transpose/ctx stage (defer 2)
            for b in range(B_LOC):
                for jp in range(N_PAIR):
                    if b > 0:
                        if jp < 2:
                            load_key_pair(b, jp + 2)
                        if jp == 0:
                            load_value(b)
                    if jp >= 2 and b + 1 < B_LOC:
                        load_key_pair(b + 1, jp - 2)
                    # score stage first: frees the previous pair's h_t tiles
                    # while this pair's kp matmuls run.
                    if sc_q:
                        emit_score_pair(*sc_q.pop(0))
                    hts, first_mm, last_mm = emit_kp_pair(b, jp)
                    sc_q.append((b, jp, hts))
                    if b == 0 and jp == 0:
                        # batch-0 bulk loads start only after the first kp
                        # matmul and run chained, so each gets full DMA
                        # bandwidth in the order the compute needs it.
                        prev = first_mm
                        for d in (load_key_pair(0, 1), load_key_pair(0, 2),
                                  load_key_pair(0, 3), load_value(0)):
                            add_dep_helper(
                                d.ins, prev.ins, sync=True,
                                reason="chain batch-0 bulk loads past startup")
                            prev = d
                    if jp == 1 and b > 0:
                        emit_softmax(b - 1)
                    if jp == 2 and b > 0:
                        emit_ctx_finish(b - 1)
                    if len(tr_q) >= 2:
                        emit_ctx_pair(*tr_q.pop(0))
                    tr_q.append((b, jp))
            # drain the pipeline
            emit_score_pair(*sc_q.pop(0))
            emit_ctx_pair(*tr_q.pop(0))
            emit_ctx_pair(*tr_q.pop(0))
            emit_softmax(B_LOC - 1)
            emit_ctx_finish(B_LOC - 1)

    nc.finalize()
    return nc


_NC_CACHE = None


def _get_nc():
    global _NC_CACHE
    if _NC_CACHE is None:
        _NC_CACHE = build_kernel()
    return _NC_CACHE


def prep_inputs(query, key, value, Wq, Wk, bias, Ws, bs):
    """Host-side shard + layout prep. Returns in_maps for the 8 cores."""
    query = np.asarray(query, dtype=np.float32)
    key = np.asarray(key, dtype=np.float32)
    value = np.asarray(value, dtype=np.float32)
    Wq = np.asarray(Wq, dtype=np.float32)
    Wk = np.asarray(Wk, dtype=np.float32)
    bias = np.asarray(bias, dtype=np.float32)
    Ws = np.asarray(Ws, dtype=np.float32)

    # wkT[p, k, o] = Wk[o, k*128+p]
    wkT = np.ascontiguousarray(
        Wk.T.reshape(N_HC, 128, H).transpose(1, 0, 2)).astype(NP_BF16)
    ws_p = np.ascontiguousarray(Ws.reshape(N_OC, 128).T).astype(NP_BF16)
    qb_full = (query[:, 0, :] @ Wq.T + bias).astype(np.float32)  # [B, H]
    ident = np.eye(32, dtype=np.float32)

    in_maps = []
    for c in range(N_CORES):
        sl = slice(c * B_LOC, (c + 1) * B_LOC)
        # keyT[b, jp, p, k, vv] = key[b, jp*PAIR+vv, k*128+p]
        keyT = (key[sl].reshape(B_LOC, N_PAIR, PAIR, N_HC, 128)
                .transpose(0, 1, 4, 3, 2).astype(NP_BF16))
        # val[b, p, c, o] = value[b, c*128+p, o]
        val_b = (value[sl].reshape(B_LOC, N_CC, 128, H)
                 .transpose(0, 2, 1, 3).astype(NP_BF16))
        qb = np.ascontiguousarray(
            qb_full[sl].reshape(B_LOC, N_OC, 128).transpose(2, 0, 1)
            .reshape(128, B_LOC * N_OC))
        in_maps.append({
            "keyT": np.ascontiguousarray(keyT),
            "value": np.ascontiguousarray(val_b),
            "wkT": wkT,
            "ws": ws_p,
            "qb": qb,
            "ident": ident,
        })
    return in_maps


def kernel(query, key, value, Wq, Wk, bias, Ws, bs):
    nc = _get_nc()
    in_maps = prep_inputs(query, key, value, Wq, Wk, bias, Ws, bs)
    res = run_bass_kernel_spmd(nc, in_maps, core_ids=list(range(N_CORES)))
    context = np.empty((B, Q, H), dtype=np.float32)
    attn = np.empty((B, V), dtype=np.float32)
    for c in range(N_CORES):
        sl = slice(c * B_LOC, (c + 1) * B_LOC)
        context[sl, 0, :] = res.results[c]["ctx_out"]
        attn[sl] = res.results[c]["attn_out"]
    return context, attn


# revision 39
# speedup vs baseline: 1.4485x; 1.0204x over previous
"""Additive (Bahdanau) attention kernel for Trainium2, SPMD over 8 NeuronCores.

Problem: B=32, Q=1, V=4096, H=1024 (fp32).
  kp    = key @ Wk^T                      (B, V, H)
  h     = tanh(kp + query @ Wq^T + bias)  (B, V, H)
  score = h @ Ws^T + bs                   (B, V)
  attn  = softmax(score)                  (B, V)   [bs drops: softmax shift-invariant]
  ctx   = attn @ value                    (B, 1, H)

Sharding: data-parallel over batch, 4 batches per core, no collectives.

Per-core dataflow (matmuls bf16 with fp32 PSUM accumulation), pipelined over
v-chunk pairs (1024 keys each), with each stage deferred in the PE stream so
the TensorEngine never waits on the softmax chain:
  stage s:   kp^T pair matmuls -> tanh (ScalarE, fused qb bias) -> h_t bf16
  stage s-1: score matmuls (Ws^T @ h_t accumulated over o-chunks) -> exp of the
             pair (no max subtraction: scores are bounded, fp32 exp is exact
             enough and softmax is shift-invariant) with fused pair-sum ->
             contiguous DRAM scatter of p -> strided read-back [8, 128]
  stage s-2: PE transpose [8,128] -> [128,8] (v on partitions), cast bf16,
             16 context matmuls (p_chunk^T @ value_chunk, accumulated in PSUM
             across the batch)
  batch end: sum of pair-sums -> reciprocal -> attn = p * rs (DMA out);
             ctx = psum * rs (DMA out).
"""

import numpy as np
import ml_dtypes

import concourse.bacc as bacc
import concourse.bass as bass
import concourse.mybir as mybir
import concourse.tile as tile
from concourse.bass_utils import run_bass_kernel_spmd

BF16 = mybir.dt.bfloat16
F32 = mybir.dt.float32
NP_BF16 = ml_dtypes.bfloat16

N_CORES = 8
B, Q, V, H = 32, 1, 4096, 1024
B_LOC = B // N_CORES          # 4 batches per core
VC = 512                      # v-chunk width for pass 1 (PSUM bank = 512 fp32)
N_VC = V // VC                # 8 v-chunks
N_OC = H // 128               # 8 o-chunks (output feature chunks)
N_HC = H // 128               # 8 h-chunks (contraction chunks)
N_CC = V // 128               # 32 v-chunks of 128 for the context matmul
PAIR = 2 * VC                 # v-chunk pair; also the keyT streaming quantum
N_PAIR = V // PAIR            # 4 pairs per batch


def build_kernel():
    nc = bacc.Bacc("TRN2", target_bir_lowering=False, debug=False,
                   num_devices=N_CORES)

    # pre-tiled host layouts: partition lines are fully contiguous in DRAM
    keyT_d = nc.declare_dram_parameter(
        "keyT", [B_LOC, N_PAIR, 128, N_HC, PAIR], BF16, isOutput=False)
    val_d = nc.declare_dram_parameter(
        "value", [B_LOC, 128, N_CC, H], BF16, isOutput=False)
    wkT_d = nc.declare_dram_parameter(
        "wkT", [128, N_HC, H], BF16, isOutput=False)
    ws_d = nc.declare_dram_parameter("ws", [128, N_OC], BF16, isOutput=False)
    qb_d = nc.declare_dram_parameter("qb", [128, B_LOC * N_OC], F32, isOutput=False)
    id_d = nc.declare_dram_parameter("ident", [32, 32], F32, isOutput=False)
    ctx_d = nc.declare_dram_parameter("ctx_out", [B_LOC, H], F32, isOutput=True)
    attn_d = nc.declare_dram_parameter("attn_out", [B_LOC, V], F32, isOutput=True)

    with tile.TileContext(nc) as tc:
        with (
            tc.tile_pool(name="weights", bufs=1) as wpool,
            tc.tile_pool(name="keyq", bufs=2) as kpool,
            tc.tile_pool(name="vals", bufs=1) as vpool,
            tc.tile_pool(name="ht", bufs=20) as htpool,
            tc.tile_pool(name="rows", bufs=1) as rowpool,
            tc.tile_pool(name="small", bufs=2) as spool,
            tc.tile_pool(name="stats", bufs=4) as stpool,
            tc.tile_pool(name="pdram", bufs=2, space="DRAM") as dpool,
            tc.tile_pool(name="pskp", bufs=3, space="PSUM") as pskp,
            tc.tile_pool(name="pssc", bufs=2, space="PSUM") as pssc,
            tc.tile_pool(name="trctx", bufs=3, space="PSUM") as trctx,
        ):
            # ---- persistent weights. wk on the gpsimd queue (the scalar
            # queue is blocked by ACT_TABLE_LOAD at startup, the sync queue
            # carries the startup-critical first keyT pair); split per
            # o-chunk so the first kp matmuls only wait for 256KB. ----
            wk_sb = wpool.tile([128, N_HC, H], BF16, tag="wk")
            for i in range(N_OC):
                nc.gpsimd.dma_start(wk_sb[:, :, i * 128:(i + 1) * 128],
                                    wkT_d[:, :, i * 128:(i + 1) * 128])
            ws_sb = wpool.tile([128, N_OC], BF16, tag="ws")
            nc.scalar.dma_start(ws_sb[:], ws_d[:])
            qb_sb = wpool.tile([128, B_LOC * N_OC], F32, tag="qb")
            nc.scalar.dma_start(qb_sb[:], qb_d[:])
            id_sb = wpool.tile([32, 32], F32, tag="ident")
            nc.scalar.dma_start(id_sb[:], id_d[:])
            ones_sb = wpool.tile([128, 1], F32, tag="ones")
            nc.gpsimd.memset(ones_sb[:], 1.0)

            key_tiles = {}   # (b, jp) -> tile [128, N_HC, PAIR]
            val_tiles = {}   # b -> tile [128, N_CC, H]

            def load_key_pair(b, jp):
                t = kpool.tile([128, N_HC, PAIR], BF16, tag="keyq",
                               name="keyq", bufs=3)
                d = nc.sync.dma_start(t[:], keyT_d[b, jp])
                key_tiles[(b, jp)] = t
                return d

            def load_value(b):
                t = vpool.tile([128, N_CC, H], BF16, tag="val", name="val")
                d = nc.gpsimd.dma_start(t[:], val_d[b])
                val_tiles[b] = t
                return d

            # ---- per-batch state ----
            score_sb = {}    # b -> [1, V] f32
            p_row = {}       # b -> [1, V] f32 unnormalized exp(score)
            p_sums = {}      # b -> [1, N_PAIR] f32 per-pair exp sums
            pd_dram = {}     # b -> [32, 128] f32 DRAM scratch (v = c*128 + p)
            prs_sb = {}      # (b, jp) -> [8, 128] f32 read-back (partition 0)
            pbf_sb = {}      # b -> [128, N_CC] bf16 (v on partitions)
            ctx_ps = {}      # b -> two [1, 512] psum halves
            rs_st = {}       # b -> [1, 1] f32 reciprocal of softmax sum

            def batch_state(b):
                if b in score_sb:
                    return
                score_sb[b] = rowpool.tile([1, V], F32, tag="score",
                                           name="score", bufs=2)
                p_row[b] = rowpool.tile([1, V], F32, tag="p", name="p", bufs=1)
                p_sums[b] = stpool.tile([1, N_PAIR], F32, tag="psums",
                                        name="psums")
                pd_dram[b] = dpool.tile([32, 128], F32, tag="pd", name="pd")
                pbf_sb[b] = spool.tile([128, N_CC], BF16, tag="pbf", name="pbf")

            def emit_kp_pair(b, jp):
                """Pair of v-chunks: 128 matmuls + 16 tanh -> h_t (i, jj)."""
                kt = key_tiles[(b, jp)]
                hts = {}
                first_mm = last_mm = None
                for i in range(N_OC):
                    pss = [pskp.tile([128, VC], F32, tag="pskp", name="pskp")
                           for _ in range(2)]
                    for k in range(N_HC):
                        for jj in range(2):
                            last_mm = nc.tensor.matmul(
                                pss[jj][:],
                                wk_sb[:, k, i * 128:(i + 1) * 128],
                                kt[:, k, jj * VC:(jj + 1) * VC],
                                start=(k == 0), stop=(k == N_HC - 1))
                            if first_mm is None:
                                first_mm = last_mm
                    for jj in range(2):
                        ht = htpool.tile([128, VC], BF16, tag="ht", name="ht")
                        nc.scalar.activation(
                            ht[:], pss[jj][:], mybir.ActivationFunctionType.Tanh,
                            bias=qb_sb[:, b * N_OC + i:b * N_OC + i + 1])
                        hts[(i, jj)] = ht
                return hts, first_mm, last_mm

            def emit_score_pair(b, jp, hts):
                """Score matmuls, then exp + pair-sum + DRAM scatter + strided
                read-back for the pair (all overlapped with later kp pairs)."""
                batch_state(b)
                for jj in range(2):
                    j = 2 * jp + jj
                    ps = pssc.tile([1, VC], F32, tag="pssc", name="pssc")
                    for i in range(N_OC):
                        nc.tensor.matmul(
                            ps[:], ws_sb[:, i:i + 1], hts[(i, jj)][:],
                            start=(i == 0), stop=(i == N_OC - 1))
                    nc.vector.tensor_copy(
                        score_sb[b][0:1, j * VC:(j + 1) * VC], ps[:])
                seg = slice(jp * PAIR, (jp + 1) * PAIR)
                rows = slice(8 * jp, 8 * (jp + 1))
                nc.scalar.activation(
                    p_row[b][0:1, seg], score_sb[b][0:1, seg],
                    mybir.ActivationFunctionType.Exp,
                    accum_out=p_sums[b][0:1, jp:jp + 1])
                nc.scalar.dma_start(
                    pd_dram[b][rows, :].rearrange("c p -> (c p)")[None, :],
                    p_row[b][0:1, seg])
                prs = spool.tile([8, 128], F32, tag="prs", name="prs", bufs=3)
                nc.scalar.dma_start(prs[:], pd_dram[b][rows, :])
                prs_sb[(b, jp)] = prs

            acc_sb = {}      # b -> [128, H] f32 (DVE context accumulator)
            pf_sb = {}       # b -> [128, N_CC] f32 (p with v on partitions)

            def emit_ctx_pair(b, jp):
                """PE transpose of the pair's p to [128, 8] (v on partitions).

                Last batch: 16 context matmuls on PE (accumulated in PSUM) -
                it's the kernel tail, PE finishes it fastest. Other batches:
                the context reduction runs on the otherwise-idle VectorE
                (acc += p_c * value_c per 128-chunk), freeing ~12us of PE per
                batch; a single ones-vector matmul in emit_ctx_finish folds
                the 128 partitions.
                """
                rows = slice(8 * jp, 8 * (jp + 1))
                pst = trctx.tile([128, 8], F32, tag="pstr", name="pstr",
                                 bufs=1)
                nc.tensor.transpose(pst[:], prs_sb[(b, jp)][:],
                                    id_sb[0:8, 0:8])
                vt = val_tiles[b]
                if b == B_LOC - 1:
                    nc.vector.tensor_copy(pbf_sb[b][:, rows], pst[:])
                    if b not in ctx_ps:
                        ctx_ps[b] = [trctx.tile([1, 512], F32, tag="psctx",
                                                name="psctx", bufs=2)
                                     for _ in range(2)]
                    for half in range(2):
                        o0 = half * 512
                        for cc in range(8):
                            c = jp * 8 + cc
                            nc.tensor.matmul(
                                ctx_ps[b][half][:],
                                pbf_sb[b][:, c:c + 1],
                                vt[:, c, o0:o0 + 512],
                                start=(jp == 0 and cc == 0),
                                stop=(jp == N_PAIR - 1 and cc == 7))
                    return
                if b not in pf_sb:
                    pf_sb[b] = spool.tile([128, N_CC], F32, tag="pf32",
                                          name="pf32")
                    acc_sb[b] = spool.tile([128, H], F32, tag="acc",
                                           name="acc", bufs=1)
                nc.vector.tensor_copy(pf_sb[b][:, rows], pst[:])
                for cc in range(8):
                    c = jp * 8 + cc
                    if jp == 0 and cc == 0:
                        nc.vector.tensor_scalar_mul(
                            acc_sb[b][:], vt[:, 0, :], pf_sb[b][:, 0:1])
                    else:
                        nc.vector.scalar_tensor_tensor(
                            acc_sb[b][:], vt[:, c, :], pf_sb[b][:, c:c + 1],
                            acc_sb[b][:], mybir.AluOpType.mult,
                            mybir.AluOpType.add)

            def emit_softmax(b):
                sm = stpool.tile([1, 1], F32, tag="sm", name="sm")
                nc.vector.reduce_sum(
                    sm[:], p_sums[b][:], axis=mybir.AxisListType.X)
                rs = stpool.tile([1, 1], F32, tag="rs", name="rs")
                nc.vector.reciprocal(rs[:], sm[:])
                rs_st[b] = rs
                attn_f = rowpool.tile([1, V], F32, tag="score", name="attnf",
                                      bufs=2)
                nc.vector.tensor_scalar_mul(attn_f[:], p_row[b][:], rs[:])
                nc.scalar.dma_start(attn_d[b:b + 1, :], attn_f[0:1, :])

            def emit_ctx_finish(b):
                if b != B_LOC - 1:
                    # fold the 128 partitions of the DVE accumulator with a
                    # single ones-vector matmul per half
                    ctx_ps[b] = [trctx.tile([1, 512], F32, tag="psctx",
                                            name="psctx", bufs=2)
                                 for _ in range(2)]
                    for half in range(2):
                        nc.tensor.matmul(
                            ctx_ps[b][half][:], ones_sb[:],
                            acc_sb[b][:, half * 512:(half + 1) * 512],
                            start=True, stop=True)
                ctx_sb = spool.tile([1, H], F32, tag="ctxsb", name="ctxsb", bufs=1)
                for half in range(2):
                    nc.vector.tensor_scalar_mul(
                        ctx_sb[0:1, half * 512:(half + 1) * 512],
                        ctx_ps[b][half][:], rs_st[b][:])
                nc.scalar.dma_start(ctx_d[b:b + 1, :], ctx_sb[0:1, :])

            # ---- main emission loop; PE program order is emission order.
            # Stages are deferred so the PE stream never waits on the
            # softmax/reshape chain of the same pair. ----
            from concourse.tile_rust import add_dep_helper

            load_key_pair(0, 0)
            sc_q = []    # pending (b, jp, hts) for score stage (defer 1)
            tr_q = []    # pending (b, jp) for transpose/ctx stage (defer 2)
            for b in range(B_LOC):
                for jp in range(N_PAIR):
                    if b > 0:
                        if jp < 2:
                            load_key_pair(b, jp + 2)
                        if jp == 0:
                            load_value(b)
                    if jp >= 2 and b + 1 < B_LOC:
                        load_key_pair(b + 1, jp - 2)
                    # score stage first: frees the previous pair's h_t tiles
                    # while this pair's kp matmuls run.
                    if sc_q:
                        emit_score_pair(*sc_q.pop(0))
                    hts, first_mm, last_mm = emit_kp_pair(b, jp)
                    sc_q.append((b, jp, hts))
                    if b == 0 and jp == 0:
                        # batch-0 bulk loads start only after the first kp
                        # matmul and run chained, so each gets full DMA
                        # bandwidth in the order the compute needs it.
                        prev = first_mm
                        for d in (load_key_pair(0, 1), load_key_pair(0, 2),
                                  load_key_pair(0, 3), load_value(0)):
                            add_dep_helper(
                                d.ins, prev.ins, sync=True,
                                reason="chain batch-0 bulk loads past startup")
                            prev = d
                    if jp == 1 and b > 0:
                        emit_softmax(b - 1)
                    if jp == 2 and b > 0:
                        emit_ctx_finish(b - 1)
                    if len(tr_q) >= 2:
                        emit_ctx_pair(*tr_q.pop(0))
                    tr_q.append((b, jp))
            # drain the pipeline
            emit_score_pair(*sc_q.pop(0))
            emit_ctx_pair(*tr_q.pop(0))
            emit_ctx_pair(*tr_q.pop(0))
            emit_softmax(B_LOC - 1)
            emit_ctx_finish(B_LOC - 1)

    nc.finalize()
    return nc


_NC_CACHE = None


def _get_nc():
    global _NC_CACHE
    if _NC_CACHE is None:
        _NC_CACHE = build_kernel()
    return _NC_CACHE


def prep_inputs(query, key, value, Wq, Wk, bias, Ws, bs):
    """Host-side shard + layout prep. Returns in_maps for the 8 cores."""
    query = np.asarray(query, dtype=np.float32)
    key = np.asarray(key, dtype=np.float32)
    value = np.asarray(value, dtype=np.float32)
    Wq = np.asarray(Wq, dtype=np.float32)
    Wk = np.asarray(Wk, dtype=np.float32)
    bias = np.asarray(bias, dtype=np.float32)
    Ws = np.asarray(Ws, dtype=np.float32)

    # wkT[p, k, o] = Wk[o, k*128+p]
    wkT = np.ascontiguousarray(
        Wk.T.reshape(N_HC, 128, H).transpose(1, 0, 2)).astype(NP_BF16)
    ws_p = np.ascontiguousarray(Ws.reshape(N_OC, 128).T).astype(NP_BF16)
    qb_full = (query[:, 0, :] @ Wq.T + bias).astype(np.float32)  # [B, H]
    ident = np.eye(32, dtype=np.float32)

    in_maps = []
    for c in range(N_CORES):
        sl = slice(c * B_LOC, (c + 1) * B_LOC)
        # keyT[b, jp, p, k, vv] = key[b, jp*PAIR+vv, k*128+p]
        keyT = (key[sl].reshape(B_LOC, N_PAIR, PAIR, N_HC, 128)
                .transpose(0, 1, 4, 3, 2).astype(NP_BF16))
        # val[b, p, c, o] = value[b, c*128+p, o]
        val_b = (value[sl].reshape(B_LOC, N_CC, 128, H)
                 .transpose(0, 2, 1, 3).astype(NP_BF16))
        qb = np.ascontiguousarray(
            qb_full[sl].reshape(B_LOC, N_OC, 128).transpose(2, 0, 1)
            .reshape(128, B_LOC * N_OC))
        in_maps.append({
            "keyT": np.ascontiguousarray(keyT),
            "value": np.ascontiguousarray(val_b),
            "wkT": wkT,
            "ws": ws_p,
            "qb": qb,
            "ident": ident,
        })
    return in_maps


def kernel(query, key, value, Wq, Wk, bias, Ws, bs):
    nc = _get_nc()
    in_maps = prep_inputs(query, key, value, Wq, Wk, bias, Ws, bs)
    res = run_bass_kernel_spmd(nc, in_maps, core_ids=list(range(N_CORES)))
    context = np.empty((B, Q, H), dtype=np.float32)
    attn = np.empty((B, V), dtype=np.float32)
    for c in range(N_CORES):
        sl = slice(c * B_LOC, (c + 1) * B_LOC)
        context[sl, 0, :] = res.results[c]["ctx_out"]
        attn[sl] = res.results[c]["attn_out"]
    return context, attn
